# revision 1
# baseline (speedup 1.0000x reference)
"""Trainium2 Bass kernel for DNN-IVA (15-iteration ISS + per-frame MLP mask net).

Sharding: data-parallel over B (4 ways) x T (2 ways) = 8 cores.
Each core handles one batch element's half of the time frames.  The only
cross-core coupling is the per-iteration reduction over T (the ISS statistics),
reformulated so each iteration needs exactly ONE tiny pair-AllReduce (20 KB).

Math reformulation (validated vs reference): per iteration, both ISS source
steps depend on the big (C,F,T) tensors only through 8 per-(f) reductions
  q0..q3 = sum_t w_c * |Y_i|^2,   q4..q7 = sum_t w_c * Re/Im(Y1 conj(Y0))
after which the source-step updates collapse to a per-frequency 2x2 complex
matrix A applied to the two channel rows:  Y'' = A Y.

On-chip layout: f on partitions (5 chunks of 128; chunk 4 has 1 valid lane),
t on the free dimension.  Products+reductions fused via tensor_tensor_reduce;
the 2x2 apply uses scalar_tensor_tensor with per-partition coefficient APs.

Host I/O path: all per-core inputs are packed into ONE fp16 buffer (one
device_put over the axon tunnel), the output is ONE fp16 buffer per core
(one fetch).  The jitted shard_map executable is cached across calls, and
the previous call's (fully-overwritten) output buffer is donated back as
the next call's output allocation so no zero-buffer is ever transferred.
"""

import os

import numpy as np

import concourse.bass as bass
import concourse.tile as tile
from concourse import bacc, mybir, masks

B, T, C, F, U = 4, 1000, 2, 513, 256
N_ITER = 15
EPS = 1e-6
N_CORES = 8
TSPLIT = 2
TL = T // TSPLIT          # 500 local frames per core
NJ = 5                    # f chunks of 128 (last has 1 valid row)
FSZ = [128, 128, 128, 128, 1]
TT_SIZES = [128, 128, 128, 116]   # t tiles covering TL=500 for load/store
FP = mybir.dt.float32
F16 = mybir.dt.float16
BF = mybir.dt.bfloat16
AL = mybir.AluOpType
AF = mybir.ActivationFunctionType

# packed fp16 input layout (per core).  The mask-net weights are sharded
# 8 ways across cores and AllGather'ed on device (cheap NeuronLink hop)
# instead of being broadcast over the slow host->device tunnel.
NXV = TL * C * F                  # 513000 elems per plane
NW = F * U                        # 131328
WTOT = 2 * NW + U + F             # 263425 packed weight elems


def _wsh_for(g):
    """Per-core weight-shard elems for a g-core group (mult of 4)."""
    return ((WTOT + g - 1) // g + 3) // 4 * 4


def _per_for(g):
    return 2 * NXV + _wsh_for(g)


WSH = _wsh_for(8)                 # 32932
OFF_XR = 0
OFF_XI = OFF_XR + NXV
OFF_WS = OFF_XI + NXV
PER = OFF_WS + WSH                # fp16 elems per core (8-core layout)
# offsets within the gathered 8*WSH weight buffer
GW1 = 0
GW2 = GW1 + NW
GB1 = GW2 + NW
GB2 = GB1 + U
OLEN = 2 * C * TL * F             # packed output: (p, c, t, f)
# int8 output mode: quantized (p,c,t,f) int8 + per-(f,j,c) fp16 scales tail
OUT8 = os.environ.get("KOUT8", "1") == "1"
NSC = 128 * NJ * C                # shipped scale slots (fp16)
SCB = 2 * NSC                     # scale tail bytes
OLEN8 = OLEN + SCB                # int8 elems per core
MAGIC = 12582912.0                # 1.5*2^23: fp32 round-to-nearest trick

_CACHED = {}


def _fslice(tile_ap, j, cols):
    """AP for f-chunk j of a [128, NJ*TL]-shaped plane (cols=TL), valid lanes only."""
    return tile_ap[0 : FSZ[j], j * cols : (j + 1) * cols]


def _build(g=N_CORES, pairs=None, wg=None):
    """pairs: explicit 2-core collective groups (default: all g//2 pairs).
    wg: weight-shard count (default g; with explicit pairs, 2)."""
    nc = bacc.Bacc("TRN2", target_bir_lowering=False, debug=False,
                   num_devices=g)
    if wg is None:
        wg = g if pairs is None else 2
    pk_d = nc.dram_tensor("pk", [_per_for(wg)], F16, kind="ExternalInput").ap()
    if OUT8:
        po_d = nc.dram_tensor("po", [OLEN8], mybir.dt.int8,
                              kind="ExternalOutput").ap()
    else:
        po_d = nc.dram_tensor("po", [OLEN], F16, kind="ExternalOutput").ap()
    with tile.TileContext(nc) as tc:
        _body(nc, tc, pk_d, po_d, g, pairs, wg)
    nc.compile()
    return nc


def _body(nc, tc, pk_d, po_d, g=N_CORES, cc_pairs=None, wg=None):
    if wg is None:
        wg = g if cc_pairs is None else 2
    if cc_pairs is None:
        cc_pairs = [[2 * i, 2 * i + 1] for i in range(g // 2)]
    PLANE = NJ * TL
    xr_d = pk_d[OFF_XR : OFF_XR + NXV].rearrange("(t c f) -> t c f", c=C, f=F)
    xi_d = pk_d[OFF_XI : OFF_XI + NXV].rearrange("(t c f) -> t c f", c=C, f=F)
    yo_d = po_d[0:OLEN].rearrange("(p c t f) -> p c t f", p=2, c=C, t=TL, f=F)
    with (
        tc.tile_pool(name="state", bufs=1) as st,
        tc.tile_pool(name="scr", bufs=3) as scr,
        tc.tile_pool(name="feat", bufs=3) as featp,
        tc.tile_pool(name="hpool", bufs=2) as hp,
        tc.tile_pool(name="small", bufs=12) as sm,
        tc.tile_pool(name="coef", bufs=2) as cf,
        tc.tile_pool(name="psA", bufs=2, space="PSUM") as psA,
        tc.tile_pool(name="psB", bufs=2, space="PSUM") as psB,
        tc.tile_pool(name="dram", bufs=2, space="DRAM") as dram,
        tc.tile_pool(name="outp", bufs=3) as outp,
    ):
        # ---- persistent state -------------------------------------------
        Y = [[st.tile([128, PLANE], FP, tag=f"Y{c}{p}", name=f"Y{c}{p}") for p in range(2)]
             for c in range(C)]                       # [c][0]=re, [1]=im
        X0 = [st.tile([128, PLANE], FP, tag=f"X0{p}", name=f"X0{p}") for p in range(2)]
        A = [st.tile([128, PLANE], BF, tag=f"a{c}", name=f"a{c}") for c in range(C)]
        Wm = [st.tile([128, PLANE], BF, tag=f"w{c}", name=f"w{c}") for c in range(C)]
        W1t = st.tile([128, NJ * U], FP, tag="W1t", name="W1t")
        W2t = st.tile([128, 2 * F], FP, tag="W2t", name="W2t")
        b1t = st.tile([128, 2], FP, tag="b1t", name="b1t")
        b2t = st.tile([128, NJ], FP, tag="b2t", name="b2t")
        ident = st.tile([128, 128], FP, tag="ident", name="ident")
        id16 = st.tile([128, 128], F16, tag="id16", name="id16")
        S = st.tile([128, 8 * NJ], FP, tag="S", name="S")       # quantity-major
        PB = st.tile([128, 12 * NJ], FP, tag="PB", name="PB")    # projection-back stats

        masks.make_identity(nc, ident[:])
        nc.scalar.copy(id16[:], ident[:])

        # ---- gather weight shards on device, then load ------------------
        wsh_g = _wsh_for(wg)
        wgroups = [list(range(g))] if wg == g else cc_pairs
        wg_i = dram.tile([1, wsh_g], F16, tag="wgi", name="wgi")
        wg_o = dram.tile([1, wg * wsh_g], F16, tag="wgo", name="wgo")
        nc.sync.dma_start(wg_i[:], pk_d[OFF_WS : OFF_WS + wsh_g]
                          .rearrange("(o k) -> o k", o=1))
        nc.gpsimd.collective_compute(
            "AllGather", AL.bypass,
            replica_groups=wgroups,
            ins=[wg_i.opt()], outs=[wg_o.opt()])
        wg = wg_o[:].squeeze(0)
        w1_d = wg[GW1 : GW1 + NW].rearrange("(f u) -> f u", u=U)
        w2_d = wg[GW2 : GW2 + NW].rearrange("(u f) -> u f", f=F)
        b1_d = wg[GB1 : GB1 + U]
        b2_d = wg[GB2 : GB2 + F]

        w1s = st.tile([128, NJ * U], F16, tag="w1s", name="w1s")
        w2s = st.tile([128, 2 * F], F16, tag="w2s", name="w2s")
        b1s = st.tile([128, 2], F16, tag="b1s", name="b1s")
        b2s = st.tile([128, NJ], F16, tag="b2s", name="b2s")
        for j in range(NJ):
            nc.sync.dma_start(w1s[0 : FSZ[j], j * U : (j + 1) * U],
                              w1_d[128 * j : 128 * j + FSZ[j], :])
            nc.sync.dma_start(b2s[0 : FSZ[j], j : j + 1],
                              b2_d[128 * j : 128 * j + FSZ[j]].rearrange("(p o) -> p o", o=1))
        for jc in range(2):
            nc.sync.dma_start(w2s[:, jc * F : (jc + 1) * F],
                              w2_d[128 * jc : 128 * (jc + 1), :])
            nc.sync.dma_start(b1s[:, jc : jc + 1],
                              b1_d[128 * jc : 128 * (jc + 1)].rearrange("(p o) -> p o", o=1))
        nc.scalar.copy(W1t[:], w1s[:])
        nc.scalar.copy(W2t[:], w2s[:])
        nc.scalar.copy(b1t[:], b1s[:])
        nc.scalar.copy(b2t[:], b2s[:])

        # ---- load input planes: (t,f) fp16 tiles -> PE transpose -> (f,t)
        for c in range(C):
            for p, src in ((0, xr_d), (1, xi_d)):
                for ti, th in enumerate(TT_SIZES):
                    it_t = scr.tile([128, F], F16, tag="ld", name="ld", bufs=2)
                    nc.sync.dma_start(it_t[0:th, :], src[ti * 128 : ti * 128 + th, c, :])
                    for j in range(NJ):
                        fj = FSZ[j]
                        ps = psB.tile([128, 128], F16, tag="tp16", name="tp16")
                        nc.tensor.transpose(ps[0:fj, 0:th],
                                            it_t[0:th, 128 * j : 128 * j + fj],
                                            id16[0:th, 0:th])
                        nc.scalar.copy(
                            Y[c][p][0:fj, j * TL + ti * 128 : j * TL + ti * 128 + th],
                            ps[0:fj, 0:th])
        for p in range(2):
            nc.vector.tensor_copy(X0[p][:], Y[0][p][:])

        # ---- helper groups ---------------------------------------------
        def qs(q):            # [128, NJ] AP of quantity q in S
            return S[:, q * NJ : (q + 1) * NJ]

        def mask_phase():
            for c in range(C):
                ph = [psA.tile([128, TL], FP, tag="ph", name="ph") for _ in range(2)]
                for j in range(NJ):
                    fj = FSZ[j]
                    s1 = scr.tile([128, TL], FP, tag="sq", name="sq", bufs=4)
                    s2 = scr.tile([128, TL], FP, tag="sq", name="sq", bufs=4)
                    nc.scalar.activation(s1[0:fj, :], _fslice(Y[c][0], j, TL), AF.Square)
                    nc.scalar.activation(s2[0:fj, :], _fslice(Y[c][1], j, TL), AF.Square)
                    nc.gpsimd.tensor_add(_fslice(A[c], j, TL), s1[0:fj, :], s2[0:fj, :])
                    ft = featp.tile([128, TL], FP, tag="ft", name="ft", bufs=4)
                    nc.scalar.activation(ft[0:fj, :], _fslice(A[c], j, TL), AF.Ln,
                                         bias=1.0)
                    for m in range(2):
                        nc.tensor.matmul(
                            ph[m][:, :],
                            W1t[0:fj, j * U + 128 * m : j * U + 128 * (m + 1)],
                            ft[0:fj, :],
                            start=(j == 0), stop=(j == NJ - 1))
                ht = hp.tile([128, 2 * TL], FP, tag="ht", name="ht")
                for m in range(2):
                    nc.scalar.activation(ht[:, m * TL : (m + 1) * TL], ph[m][:, :],
                                         AF.Tanh, bias=b1t[:, m : m + 1])
                for j in range(NJ):
                    fj = FSZ[j]
                    pm = psB.tile([128, TL], FP, tag="pm", name="pm")
                    for jc in range(2):
                        nc.tensor.matmul(
                            pm[0:fj, :],
                            W2t[:, jc * F + 128 * j : jc * F + 128 * j + fj],
                            ht[:, jc * TL : (jc + 1) * TL],
                            start=(jc == 0), stop=(jc == 1))
                    nc.scalar.activation(_fslice(Wm[c], j, TL), pm[0:fj, :],
                                         AF.Sigmoid, bias=b2t[0:fj, j : j + 1])

        def stats_phase():
            for j in range(NJ):
                fj = FSZ[j]
                y0r, y0i = _fslice(Y[0][0], j, TL), _fslice(Y[0][1], j, TL)
                y1r, y1i = _fslice(Y[1][0], j, TL), _fslice(Y[1][1], j, TL)
                m1 = scr.tile([128, TL], BF, tag="pp", name="pp", bufs=4)
                m2 = scr.tile([128, TL], BF, tag="pp", name="pp", bufs=4)
                pr = scr.tile([128, TL], BF, tag="pr", name="pr", bufs=2)
                nc.vector.tensor_mul(m1[0:fj, :], y1r, y0r)
                nc.vector.tensor_mul(m2[0:fj, :], y1i, y0i)
                nc.vector.tensor_add(pr[0:fj, :], m1[0:fj, :], m2[0:fj, :])
                m3 = scr.tile([128, TL], BF, tag="pp", name="pp", bufs=4)
                m4 = scr.tile([128, TL], BF, tag="pp", name="pp", bufs=4)
                pi = scr.tile([128, TL], BF, tag="pi", name="pi", bufs=2)
                nc.gpsimd.tensor_mul(m3[0:fj, :], y1i, y0r)
                nc.gpsimd.tensor_mul(m4[0:fj, :], y1r, y0i)
                nc.gpsimd.tensor_sub(pi[0:fj, :], m3[0:fj, :], m4[0:fj, :])
                srcs = [(Wm[0], _fslice(A[0], j, TL), 0),
                        (Wm[1], _fslice(A[0], j, TL), 1),
                        (Wm[0], _fslice(A[1], j, TL), 2),
                        (Wm[1], _fslice(A[1], j, TL), 3),
                        (Wm[0], pr[0:fj, :], 4), (Wm[0], pi[0:fj, :], 5),
                        (Wm[1], pr[0:fj, :], 6), (Wm[1], pi[0:fj, :], 7)]
                for wt, src_ap, q in srcs:
                    prod = scr.tile([128, TL], BF, tag="pd", name="pd", bufs=6)
                    eng = nc.vector if q % 2 == 0 else nc.gpsimd
                    eng.tensor_mul(prod[0:fj, :], _fslice(wt, j, TL), src_ap)
                    nc.vector.tensor_reduce(
                        S[0:fj, q * NJ + j : q * NJ + j + 1], prod[0:fj, :],
                        axis=mybir.AxisListType.X, op=AL.add)

        def allreduce(tile_t, ncols):
            bi = dram.tile([128, ncols], FP, tag="cin", name="cin")
            bo = dram.tile([128, ncols], FP, tag="cout", name="cout")
            nc.sync.dma_start(bi[:], tile_t[:, 0:ncols])
            nc.gpsimd.collective_compute(
                "AllReduce", AL.add,
                replica_groups=cc_pairs,
                ins=[bi.opt()], outs=[bo.opt()])
            nc.sync.dma_start(tile_t[:, 0:ncols], bo[:])

        def smalls():
            """Per-(f) coefficient algebra on [128, NJ] tiles."""
            def t():
                return sm.tile([128, NJ], FP, tag="smt", name="smt")

            def c(name):
                return cf.tile([128, NJ], FP, tag=name, name=name)
            invT = 1.0 / float(T)
            d0, r0 = t(), t()
            alpha = c("alpha")
            nc.vector.tensor_scalar(d0[:], qs(0), invT, EPS, AL.mult, AL.max)
            nc.vector.reciprocal(r0[:], d0[:])
            nc.scalar.activation(alpha[:], r0[:], AF.Sqrt)
            d1, r1 = t(), t()
            nc.vector.tensor_scalar(d1[:], qs(1), EPS, None, AL.max)
            nc.vector.reciprocal(r1[:], d1[:])
            vr = t()
            vi, nvr, nvi = c("vi"), c("nvr"), c("nvi")
            nc.vector.tensor_mul(vr[:], qs(6), r1[:])
            nc.vector.tensor_mul(vi[:], qs(7), r1[:])
            nc.vector.tensor_scalar_mul(nvr[:], vr[:], -1.0)
            nc.vector.tensor_scalar_mul(nvi[:], vi[:], -1.0)
            m2, u = t(), t()
            nc.vector.tensor_mul(m2[:], vr[:], vr[:])
            nc.vector.scalar_tensor_tensor(u[:], vi[:], 1.0, vi[:], AL.mult, AL.mult)
            nc.vector.tensor_add(m2[:], m2[:], u[:])
            # den0' = q2 - 2(vr q4 + vi q5) + m2 q0 ; den1' likewise with q6,q7,q1,q3
            def denp(qa, qb, qden, qs11):
                x1, x2, e = t(), t(), t()
                nc.vector.tensor_mul(x1[:], vr[:], qa)
                nc.vector.scalar_tensor_tensor(x2[:], vi[:], 1.0, qb, AL.mult, AL.mult)
                nc.vector.tensor_add(x1[:], x1[:], x2[:])
                nc.vector.tensor_mul(e[:], m2[:], qden)
                o = t()
                nc.vector.scalar_tensor_tensor(o[:], x1[:], -2.0, qs11, AL.mult, AL.add)
                nc.vector.tensor_add(o[:], o[:], e[:])
                return o
            den0p = denp(qs(4), qs(5), qs(0), qs(2))
            den1p = denp(qs(6), qs(7), qs(1), qs(3))
            dm, rdm = t(), t()
            nc.vector.tensor_scalar(dm[:], den0p[:], EPS, None, AL.max)
            nc.vector.reciprocal(rdm[:], dm[:])
            # v1 = alpha*((q4,-q5) - conj(v) q0) / den0p
            v1r, tA, tB = t(), t(), t()
            v1i, nv1r, nv1i = c("v1i"), c("nv1r"), c("nv1i")
            nc.vector.tensor_mul(tA[:], vr[:], qs(0))
            nc.vector.tensor_sub(tA[:], qs(4), tA[:])
            nc.vector.tensor_mul(tA[:], tA[:], alpha[:])
            nc.vector.tensor_mul(v1r[:], tA[:], rdm[:])
            nc.vector.tensor_mul(tB[:], vi[:], qs(0))
            nc.vector.tensor_sub(tB[:], tB[:], qs(5))
            nc.vector.tensor_mul(tB[:], tB[:], alpha[:])
            nc.vector.tensor_mul(v1i[:], tB[:], rdm[:])
            nc.vector.tensor_scalar_mul(nv1r[:], v1r[:], -1.0)
            nc.vector.tensor_scalar_mul(nv1i[:], v1i[:], -1.0)
            db, rb = t(), t()
            beta = c("beta")
            nc.vector.tensor_scalar(db[:], den1p[:], invT, EPS, AL.mult, AL.max)
            nc.vector.reciprocal(rb[:], db[:])
            nc.scalar.activation(beta[:], rb[:], AF.Sqrt)
            return alpha, beta, vi, nvr, nvi, v1i, nv1r, nv1i

        def apply_phase(alpha, beta, vi, nvr, nvi, v1i, nv1r, nv1i):
            for j in range(NJ):
                fj = FSZ[j]
                y0r, y0i = _fslice(Y[0][0], j, TL), _fslice(Y[0][1], j, TL)
                y1r, y1i = _fslice(Y[1][0], j, TL), _fslice(Y[1][1], j, TL)
                def c_(ct):
                    return ct[0:fj, j : j + 1]
                t1 = scr.tile([128, TL], FP, tag="ap", name="ap", bufs=4)
                y1pr = scr.tile([128, TL], FP, tag="y1p", name="y1p")
                nc.vector.scalar_tensor_tensor(t1[0:fj, :], y0r, c_(nvr), y1r,
                                               AL.mult, AL.add)
                nc.vector.scalar_tensor_tensor(y1pr[0:fj, :], y0i, c_(vi), t1[0:fj, :],
                                               AL.mult, AL.add)
                t2 = scr.tile([128, TL], FP, tag="ap", name="ap", bufs=4)
                y1pi = scr.tile([128, TL], FP, tag="y1p", name="y1p")
                nc.vector.scalar_tensor_tensor(t2[0:fj, :], y0i, c_(nvr), y1i,
                                               AL.mult, AL.add)
                nc.vector.scalar_tensor_tensor(y1pi[0:fj, :], y0r, c_(nvi), t2[0:fj, :],
                                               AL.mult, AL.add)
                s1 = scr.tile([128, TL], FP, tag="ap", name="ap", bufs=4)
                s2 = scr.tile([128, TL], FP, tag="ap", name="ap", bufs=4)
                nc.scalar.mul(s1[0:fj, :], y0r, c_(alpha))
                nc.scalar.mul(s2[0:fj, :], y0i, c_(alpha))
                t3 = scr.tile([128, TL], FP, tag="ap", name="ap", bufs=4)
                nc.vector.scalar_tensor_tensor(t3[0:fj, :], y1pr[0:fj, :], c_(nv1r),
                                               s1[0:fj, :], AL.mult, AL.add)
                nc.vector.scalar_tensor_tensor(y0r, y1pi[0:fj, :], c_(v1i),
                                               t3[0:fj, :], AL.mult, AL.add)
                t4 = scr.tile([128, TL], FP, tag="ap", name="ap", bufs=4)
                nc.vector.scalar_tensor_tensor(t4[0:fj, :], y1pi[0:fj, :], c_(nv1r),
                                               s2[0:fj, :], AL.mult, AL.add)
                nc.vector.scalar_tensor_tensor(y0i, y1pr[0:fj, :], c_(nv1i),
                                               t4[0:fj, :], AL.mult, AL.add)
                nc.scalar.mul(y1r, y1pr[0:fj, :], c_(beta))
                nc.scalar.mul(y1i, y1pi[0:fj, :], c_(beta))

        # ---- main loop ---------------------------------------------------
        n_it = int(os.environ.get("KITERS", str(N_ITER)))
        do_cc = os.environ.get("KCC", "1") == "1"
        do_pb = os.environ.get("KPB", "1") == "1"
        do_mask = os.environ.get("KMASK", "1") == "1"
        do_stats = os.environ.get("KSTATS", "1") == "1"
        do_apply = os.environ.get("KAPPLY", "1") == "1"
        for _ in range(n_it):
            if do_mask:
                mask_phase()
            if do_stats:
                stats_phase()
            if do_cc:
                allreduce(S, 8 * NJ)
            if do_apply:
                coefs = smalls()
                apply_phase(*coefs)

        # ---- projection back --------------------------------------------
        for j in ([] if not do_pb else range(NJ)):
            fj = FSZ[j]
            for c in range(C):
                pairs = [(Y[c][0], X0[0]), (Y[c][1], X0[1]),
                         (Y[c][0], X0[1]), (Y[c][1], X0[0]),
                         (Y[c][0], Y[c][0]), (Y[c][1], Y[c][1])]
                for qi, (ta, tb) in enumerate(pairs):
                    q = c * 6 + qi
                    prod = scr.tile([128, TL], FP, tag="pd2", name="pd2", bufs=4)
                    if qi >= 4:
                        nc.scalar.activation(prod[0:fj, :], _fslice(ta, j, TL),
                                             AF.Square)
                    else:
                        eng = nc.vector if qi % 2 == 0 else nc.gpsimd
                        eng.tensor_mul(prod[0:fj, :], _fslice(ta, j, TL),
                                       _fslice(tb, j, TL))
                    nc.vector.tensor_reduce(
                        PB[0:fj, q * NJ + j : q * NJ + j + 1], prod[0:fj, :],
                        axis=mybir.AxisListType.X, op=AL.add)
        if do_pb:
            allreduce(PB, 12 * NJ)

        def pbq(q):
            return PB[:, q * NJ : (q + 1) * NJ]

        for c in ([] if not do_pb else range(C)):
            g = [pbq(c * 6 + i) for i in range(6)]
            numr = sm.tile([128, NJ], FP, tag="pbs", name="pbs")
            numi = sm.tile([128, NJ], FP, tag="pbs", name="pbs")
            den = sm.tile([128, NJ], FP, tag="pbs", name="pbs")
            rc = sm.tile([128, NJ], FP, tag="pbs", name="pbs")
            cr = sm.tile([128, NJ], FP, tag=f"cr{c}", name=f"cr{c}")
            ci = sm.tile([128, NJ], FP, tag=f"ci{c}", name=f"ci{c}")
            nci = sm.tile([128, NJ], FP, tag=f"nci{c}", name=f"nci{c}")
            nc.vector.tensor_add(numr[:], g[0], g[1])
            nc.vector.tensor_sub(numi[:], g[2], g[3])
            nc.vector.tensor_add(den[:], g[4], g[5])
            nc.vector.tensor_scalar(den[:], den[:], EPS, None, AL.max)
            nc.vector.reciprocal(rc[:], den[:])
            nc.vector.tensor_mul(cr[:], numr[:], rc[:])
            nc.vector.tensor_mul(ci[:], numi[:], rc[:])
            nc.vector.tensor_scalar_mul(nci[:], ci[:], -1.0)
            for j in range(NJ):
                fj = FSZ[j]
                ycr, yci = _fslice(Y[c][0], j, TL), _fslice(Y[c][1], j, TL)
                s1 = scr.tile([128, TL], FP, tag="ap", name="ap", bufs=4)
                s2 = scr.tile([128, TL], FP, tag="ap", name="ap", bufs=4)
                tr = scr.tile([128, TL], FP, tag="ap", name="ap", bufs=4)
                nc.scalar.mul(s1[0:fj, :], ycr, cr[0:fj, j : j + 1])
                nc.scalar.mul(s2[0:fj, :], yci, cr[0:fj, j : j + 1])
                # new_re = cr*ycr - ci*yci ; new_im = cr*yci + ci*ycr
                nc.vector.scalar_tensor_tensor(tr[0:fj, :], yci, nci[0:fj, j : j + 1],
                                               s1[0:fj, :], AL.mult, AL.add)
                nc.vector.scalar_tensor_tensor(yci, ycr, ci[0:fj, j : j + 1],
                                               s2[0:fj, :], AL.mult, AL.add)
                nc.vector.tensor_copy(ycr, tr[0:fj, :])

        # ---- write output: transpose back to (t,f), DMA out -------------
        if OUT8:
            # per-(c,f) scales: max |.| over local t of re/im, shipped fp16
            mx = st.tile([128, NJ * C], FP, tag="mx", name="mx")
            sinv = st.tile([128, NJ * C], FP, tag="sinv", name="sinv")
            scf = st.tile([128, NJ * C], F16, tag="scf", name="scf")
            for c in range(C):
                for j in range(NJ):
                    fj = FSZ[j]
                    col = j * C + c
                    a1 = scr.tile([128, TL], FP, tag="ab", name="ab", bufs=4)
                    a2 = scr.tile([128, TL], FP, tag="ab", name="ab", bufs=4)
                    nc.scalar.activation(a1[0:fj, :], _fslice(Y[c][0], j, TL), AF.Abs)
                    nc.scalar.activation(a2[0:fj, :], _fslice(Y[c][1], j, TL), AF.Abs)
                    nc.vector.tensor_max(a1[0:fj, :], a1[0:fj, :], a2[0:fj, :])
                    nc.vector.tensor_reduce(mx[0:fj, col : col + 1], a1[0:fj, :],
                                            axis=mybir.AxisListType.X, op=AL.max)
            nc.vector.tensor_scalar(sinv[:], mx[:], 1e-30, None, AL.max)
            nc.vector.reciprocal(sinv[:], sinv[:])
            nc.vector.tensor_scalar_mul(sinv[:], sinv[:], 127.0)
            nc.vector.tensor_scalar_mul(scf[:], mx[:], 1.0 / 127.0)
            sc_d = po_d[OLEN : OLEN + SCB].bitcast(F16).rearrange(
                "(p k) -> p k", k=NJ * C)
            nc.sync.dma_start(sc_d, scf[:])
            for c in range(C):
                for p in range(2):
                    for ti, th in enumerate(TT_SIZES):
                        ot = outp.tile([128, F], mybir.dt.int8, tag="ot8",
                                       name="ot8")
                        for j in range(NJ):
                            fj = FSZ[j]
                            col = j * C + c
                            qt = scr.tile([128, 128], FP, tag="qt", name="qt",
                                          bufs=4)
                            nc.scalar.mul(
                                qt[0:fj, 0:th],
                                Y[c][p][0:fj, j * TL + ti * 128 : j * TL + ti * 128 + th],
                                sinv[0:fj, col : col + 1])
                            nc.vector.tensor_scalar(qt[0:fj, 0:th], qt[0:fj, 0:th],
                                                    MAGIC, -MAGIC, AL.add, AL.add)
                            ps = psB.tile([128, 128], FP, tag="tp", name="tp")
                            nc.tensor.transpose(ps[0:th, 0:fj], qt[0:fj, 0:th],
                                                ident[0:fj, 0:fj])
                            nc.scalar.copy(ot[0:th, 128 * j : 128 * j + fj],
                                           ps[0:th, 0:fj])
                        nc.sync.dma_start(yo_d[p, c, ti * 128 : ti * 128 + th, :],
                                          ot[0:th, :])
        else:
            for c in range(C):
                for p in range(2):
                    for ti, th in enumerate(TT_SIZES):
                        ot = outp.tile([128, F], F16, tag="ot", name="ot")
                        for j in range(NJ):
                            fj = FSZ[j]
                            ps = psB.tile([128, 128], FP, tag="tp", name="tp")
                            nc.tensor.transpose(
                                ps[0:th, 0:fj],
                                Y[c][p][0:fj, j * TL + ti * 128 : j * TL + ti * 128 + th],
                                ident[0:fj, 0:fj])
                            nc.scalar.copy(ot[0:th, 128 * j : 128 * j + fj],
                                           ps[0:th, 0:fj])
                        nc.sync.dma_start(yo_d[p, c, ti * 128 : ti * 128 + th, :],
                                          ot[0:th, :])


class _Exec:
    """Cached jitted shard_map executor with output-buffer donation."""

    def __init__(self, nc, devices=None, g=N_CORES):
        import jax
        from jax.sharding import Mesh, PartitionSpec, NamedSharding
        from jax.experimental.shard_map import shard_map
        from concourse.bass2jax import (
            _bass_exec_p, install_neuronx_cc_hook, partition_id_tensor,
        )
        import jax.numpy as jnp

        self.jax = jax
        self.np = np
        install_neuronx_cc_hook()
        partition_name = (nc.partition_id_tensor.name
                          if nc.partition_id_tensor else None)
        in_names, out_names, out_avals = [], [], []
        in_len = None
        for alloc in nc.m.functions[0].allocations:
            if not isinstance(alloc, mybir.MemoryLocationSet):
                continue
            name = alloc.memorylocations[0].name
            if alloc.kind == "ExternalInput":
                if name != partition_name:
                    in_names.append(name)
                    in_len = int(alloc.tensor_shape[0])
            elif alloc.kind == "ExternalOutput":
                out_names.append(name)
                out_avals.append(jax.core.ShapedArray(
                    tuple(alloc.tensor_shape), mybir.dt.np(alloc.dtype)))
        assert in_names == ["pk"] and out_names == ["po"], (in_names, out_names)
        n_params = len(in_names)
        n_outs = len(out_avals)
        all_in = list(in_names) + list(out_names)
        if partition_name is not None:
            all_in.append(partition_name)

        def _bdy(*args):
            operands = list(args)
            if partition_name is not None:
                operands.append(partition_id_tensor())
            return tuple(_bass_exec_p.bind(
                *operands,
                out_avals=tuple(out_avals),
                in_names=tuple(all_in),
                out_names=tuple(out_names),
                lowering_input_output_aliases=(),
                sim_require_finite=True,
                sim_require_nnan=True,
                nc=nc,
            ))

        if devices is None:
            devices = jax.devices()[:g]
        assert len(devices) == g
        self.devices = devices
        self.g = g
        self.per = in_len
        mesh = Mesh(np.asarray(devices), ("core",))
        self.shard = NamedSharding(mesh, PartitionSpec("core"))
        in_specs = (PartitionSpec("core"),) * (n_params + n_outs)
        out_specs = (PartitionSpec("core"),) * n_outs
        jitfn = jax.jit(
            shard_map(_bdy, mesh=mesh, in_specs=in_specs, out_specs=out_specs,
                      check_rep=False),
            donate_argnums=tuple(range(n_params, n_params + n_outs)),
            keep_unused=True,
        )
        aval = out_avals[0]
        gshape = (g * aval.shape[0],)
        gdtype = aval.dtype
        try:
            from concourse.bass2jax import fast_dispatch_compile
            pk_spec = jax.ShapeDtypeStruct((g * self.per,), np.float16,
                                           sharding=self.shard)
            do_spec = jax.ShapeDtypeStruct(gshape, gdtype, sharding=self.shard)
            self.sharded = fast_dispatch_compile(
                lambda: jitfn.lower(pk_spec, do_spec).compile())
        except Exception:
            self.sharded = jitfn
        self.mkzeros = jax.jit(lambda: jnp.zeros(gshape, gdtype),
                               out_shardings=self.shard)
        self.prev_out = None

    def run_packed(self, bufs):
        """bufs: list of g per-core np fp16 buffers (len self.per).
        Returns np (g*OLEN8,) int8 (or f16 in non-OUT8 builds)."""
        jax = self.jax
        donate = self.prev_out if self.prev_out is not None else self.mkzeros()
        shards = [jax.device_put(bufs[k], self.devices[k])
                  for k in range(self.g)]
        pk_dev = jax.make_array_from_single_device_arrays(
            (self.g * self.per,), self.shard, shards)
        (out,) = self.sharded(pk_dev, donate)
        try:
            out.copy_to_host_async()
        except Exception:
            pass
        res = np.asarray(out)
        self.prev_out = out
        return res

    def run(self, inputs):
        """Pack per-core fp16 buffers, pipelining each device_put with the
        next core's packing.  Returns np (N_CORES*OLEN,) fp16."""
        jax = self.jax
        donate = self.prev_out if self.prev_out is not None else self.mkzeros()
        data_real = np.asarray(inputs["data_real"]).reshape(N_CORES, TL, C, F)
        data_imag = np.asarray(inputs["data_imag"]).reshape(N_CORES, TL, C, F)
        wflat = np.empty(8 * WSH, np.float16)
        wflat[GW1 : GW1 + NW] = np.asarray(inputs["W1"]).reshape(NW)
        wflat[GW2 : GW2 + NW] = np.asarray(inputs["W2"]).reshape(NW)
        wflat[GB1 : GB1 + U] = np.asarray(inputs["b1"])
        wflat[GB2 : GB2 + F] = np.asarray(inputs["b2"])
        wflat[GB2 + F :] = 0
        wsh = wflat.reshape(N_CORES, WSH)
        # reuse pack buffers across calls: each call's transfer completes
        # before kernel() returns, so rewriting them next call is safe
        bufs = self.__dict__.setdefault(
            "pack_bufs", [np.empty(PER, np.float16) for _ in range(N_CORES)])
        shards = []
        for k in range(N_CORES):
            buf = bufs[k]
            buf[OFF_XR : OFF_XR + NXV].reshape(TL, C, F)[...] = data_real[k]
            buf[OFF_XI : OFF_XI + NXV].reshape(TL, C, F)[...] = data_imag[k]
            buf[OFF_WS : OFF_WS + WSH] = wsh[k]
            shards.append(jax.device_put(buf, self.devices[k]))
        pk_dev = jax.make_array_from_single_device_arrays(
            (N_CORES * PER,), self.shard, shards)
        (out,) = self.sharded(pk_dev, donate)
        try:
            out.copy_to_host_async()
        except Exception:
            pass
        # pre-fault the host output array while exec+fetch stream in the
        # background (the async transfer runs on C++ threads regardless)
        outbuf = np.empty((C, B, T, F), dtype=np.complex64)
        outbuf.fill(0)
        res = np.asarray(out)
        self.prev_out = out
        return res, outbuf


def _unpack(res, outbuf=None):
    out = outbuf if outbuf is not None else np.empty((C, B, T, F),
                                                     dtype=np.complex64)
    if OUT8:
        g = res.reshape(N_CORES, OLEN8)
        q = g[:, :OLEN].reshape(B, TSPLIT, 2, C, TL, F)
        sc = np.ascontiguousarray(g[:, OLEN:]).view(np.float16)
        sc = sc.reshape(N_CORES, 128, NJ, C)
        scale = np.empty((N_CORES, C, F), np.float32)
        for j in range(NJ):
            fj = FSZ[j]
            scale[:, :, 128 * j : 128 * j + fj] = \
                sc[:, 0:fj, j, :].transpose(0, 2, 1)
        sv = scale.reshape(B, TSPLIT, C, F)
        for b in range(B):
            for th in range(TSPLIT):
                sl = slice(th * TL, (th + 1) * TL)
                s = sv[b, th][:, None, :]              # (C,1,F)
                np.multiply(q[b, th, 0], s, out=out.real[:, b, sl, :])
                np.multiply(q[b, th, 1], s, out=out.imag[:, b, sl, :])
        return out
    g = res.reshape(B, TSPLIT, 2, C, TL, F)   # b, th, p, c, t, f
    for b in range(B):
        for th in range(TSPLIT):
            sl = slice(th * TL, (th + 1) * TL)
            out.real[:, b, sl, :] = g[b, th, 0]
            out.imag[:, b, sl, :] = g[b, th, 1]
    return out


def kernel(**inputs):
    if "ex" not in _CACHED:
        _CACHED["nc"] = _build()
        _CACHED["ex"] = _Exec(_CACHED["nc"])
    ex = _CACHED["ex"]
    res, outbuf = ex.run(inputs)
    return _unpack(res, outbuf)


if __name__ == "__main__":
    rng = np.random.default_rng(0)
    ins = {
        "data_real": rng.standard_normal((B, T, C, F), dtype=np.float32),
        "data_imag": rng.standard_normal((B, T, C, F), dtype=np.float32),
        "ilens": np.full((B,), T, dtype=np.int32),
        "W1": rng.standard_normal((F, U), dtype=np.float32) / np.sqrt(F),
        "b1": np.zeros((U,), dtype=np.float32),
        "W2": rng.standard_normal((U, F), dtype=np.float32) / np.sqrt(U),
        "b2": np.zeros((F,), dtype=np.float32),
    }
    out = kernel(**ins)
    print("kernel ran", out.shape, out.dtype, np.abs(out).mean())



# revision 2
# speedup vs baseline: 1.2909x; 1.2909x over previous
"""Trainium2 Bass kernel for DNN-IVA (15-iteration ISS + per-frame MLP mask net).

v2: demixing-matrix output + single-put broadcast input.

Key reformulation: every ISS source-step is LINEAR in the spectrogram X
(Y <- A Y with a per-(b,f) 2x2 complex A; projection-back is a diagonal
scale).  So the final output is  out = M(b,f) . X  with M a tiny per-(b,f)
2x2 complex matrix.  The device accumulates M alongside Y (a few dozen
[128,NJ]-sized vector ops per iteration) and ships ONLY M (160 KB) back;
the host (which already holds X at full precision) applies M in ~40 ms.
This removes the 8 MB quantized output fetch entirely AND its int8 error.

Host->device traffic is ONE ~8.8 MB int8 device_put to core 0: the packed
buffer holds all 8 cores' per-core segments (int8-quantized X slices +
fp32 scales + an 1/8 weight shard each).  Cores 1-7 receive persistent
on-device zero buffers (no transfer).  In-kernel, a ReduceScatter(add)
over all 8 cores delivers segment k to core k (zeros + core0's data =
core0's data), then an AllGather reassembles the full mask-net weights
from the 8 shards.  The axon tunnel moves 8.8 MB in + 160 KB out per call
instead of the 17 MB + 8.2 MB of the per-core-put scheme.

Compute sharding: data-parallel over B (4) x T (2) = 8 cores, per-iteration
ISS statistics pair-AllReduced over the T halves (20 KB), as before.
On-chip layout: f on partitions (5 chunks of 128), t on the free dim.
"""

import os

import numpy as np

import concourse.bass as bass
import concourse.tile as tile
from concourse import bacc, mybir, masks

B, T, C, F, U = 4, 1000, 2, 513, 256
N_ITER = 15
EPS = 1e-6
N_CORES = 8
TSPLIT = 2
TL = T // TSPLIT          # 500 local frames per core
NJ = 5                    # f chunks of 128 (last has 1 valid row)
FSZ = [128, 128, 128, 128, 1]
TT_SIZES = [128, 128, 128, 116]   # t tiles covering TL=500 for load/store
FP = mybir.dt.float32
F16 = mybir.dt.float16
BF = mybir.dt.bfloat16
I8 = mybir.dt.int8
I32 = mybir.dt.int32
AL = mybir.AluOpType
AF = mybir.ActivationFunctionType

# ---- packed int8 input layout --------------------------------------------
# per-core segment: [X re int8 | X im int8 | scales f32 | weight shard]
NXB = TL * C * F                  # 513000 bytes per plane
OFF_SC = 2 * NXB                  # 1026000
SCB = C * F * 4                   # 4104 bytes of fp32 scales
OFF_WS = OFF_SC + SCB             # 1030104
# weight blob: W1 f16 | W2 f16 | b1 f32 | b2 f32 (byte offsets)
NW = F * U                        # 131328 elems
WB_W1 = 0
WB_W2 = WB_W1 + 2 * NW            # 262656
WB_B1 = WB_W2 + 2 * NW            # 525312
WB_B2 = WB_B1 + 4 * U             # 526336
WBLOB = WB_B2 + 4 * F             # 528388 bytes
WSHB = ((WBLOB + N_CORES - 1) // N_CORES + 3) // 4 * 4   # 66052 per core
SEG_B = OFF_WS + WSHB             # 1096156 bytes per core
SEG_H = SEG_B // 2                # 548078 f16 units
PK_B = N_CORES * SEG_B            # 8769248
PK_H = PK_B // 2                  # 4384624 f16 units per-core external input
WSH_H = WSHB // 2                 # 33026
OLEN_M = N_CORES * 128 * 8 * NJ   # 40960 fp32 output (all cores' M blocks)
# Quantization level: values in [-QLVL, QLVL].  Coarser than int8's 127 on
# purpose: the tunnel compresses, so lower payload entropy = fewer wire
# bytes, while rel-err stays ~9e-3 (tolerance 2e-2).  Device side is
# unchanged (plain int8 bytes + shipped scales).
QLVL = 45.0

_CACHED = {}


def _fslice(tile_ap, j, cols):
    """AP for f-chunk j of a [128, NJ*TL]-shaped plane (cols=TL), valid lanes only."""
    return tile_ap[0 : FSZ[j], j * cols : (j + 1) * cols]


def _build(g=N_CORES):
    nc = bacc.Bacc("TRN2", target_bir_lowering=False, debug=False,
                   num_devices=g)
    pk_d = nc.dram_tensor("pk", [PK_H], F16, kind="ExternalInput").ap()
    po_d = nc.dram_tensor("po", [OLEN_M], FP, kind="ExternalOutput").ap()
    with tile.TileContext(nc) as tc:
        _body(nc, tc, pk_d, po_d, g)
    nc.compile()
    return nc


def _body(nc, tc, pk_d, po_d, g=N_CORES):
    cc_pairs = [[2 * i, 2 * i + 1] for i in range(g // 2)]
    cc_all = [list(range(g))]
    PLANE = NJ * TL
    with (
        tc.tile_pool(name="state", bufs=1) as st,
        tc.tile_pool(name="scr", bufs=3) as scr,
        tc.tile_pool(name="feat", bufs=3) as featp,
        tc.tile_pool(name="hpool", bufs=2) as hp,
        tc.tile_pool(name="small", bufs=12) as sm,
        tc.tile_pool(name="coef", bufs=2) as cf,
        tc.tile_pool(name="psA", bufs=2, space="PSUM") as psA,
        tc.tile_pool(name="psB", bufs=2, space="PSUM") as psB,
        tc.tile_pool(name="dram", bufs=1, space="DRAM") as dram,
    ):
        # ---- persistent state -------------------------------------------
        Y = [[st.tile([128, PLANE], FP, tag=f"Y{c}{p}", name=f"Y{c}{p}") for p in range(2)]
             for c in range(C)]                       # [c][0]=re, [1]=im
        X0 = [st.tile([128, PLANE], FP, tag=f"X0{p}", name=f"X0{p}") for p in range(2)]
        A = [st.tile([128, PLANE], BF, tag=f"a{c}", name=f"a{c}") for c in range(C)]
        Wm = [st.tile([128, PLANE], BF, tag=f"w{c}", name=f"w{c}") for c in range(C)]
        W1t = st.tile([128, NJ * U], FP, tag="W1t", name="W1t")
        W2t = st.tile([128, 2 * F], FP, tag="W2t", name="W2t")
        b1t = st.tile([128, 2], FP, tag="b1t", name="b1t")
        b2t = st.tile([128, NJ], FP, tag="b2t", name="b2t")
        sct = st.tile([128, NJ * C], FP, tag="sct", name="sct")
        Mt = st.tile([128, 8 * NJ], FP, tag="Mt", name="Mt")
        id16 = st.tile([128, 128], F16, tag="id16", name="id16")
        ident = st.tile([128, 128], FP, tag="ident", name="ident")
        S = st.tile([128, 8 * NJ], FP, tag="S", name="S")       # quantity-major
        PB = st.tile([128, 12 * NJ], FP, tag="PB", name="PB")    # projection-back stats

        masks.make_identity(nc, ident[:])
        nc.scalar.copy(id16[:], ident[:])

        def Mv(k, c, p):
            q = (k * 2 + c) * 2 + p
            return Mt[:, q * NJ : (q + 1) * NJ]

        nc.vector.memset(Mt[:], 0.0)
        nc.vector.memset(Mv(0, 0, 0), 1.0)
        nc.vector.memset(Mv(1, 1, 0), 1.0)

        # ---- scatter the single-put packed input to all cores -----------
        # AllToAll (pure bypass, bit-exact): core 0's input holds all 8
        # per-core segments, cores 1-7 hold zeros; core k's output slot 0
        # is core 0's segment k == its own data.  (Reduce-type collectives
        # run their ALU in fp32 here and corrupt raw byte payloads.)
        gi = dram.tile([1, PK_H], F16, tag="gi", name="gi")
        go = dram.tile([1, PK_H], F16, tag="go", name="go")
        nc.sync.dma_start(gi[:], pk_d.rearrange("(o k) -> o k", o=1))
        nc.gpsimd.collective_compute(
            "AllToAll", AL.bypass,
            replica_groups=cc_all,
            ins=[gi.opt()], outs=[go.opt()])
        seg = go[:].squeeze(0)[0:SEG_H]              # [SEG_H] f16 AP
        x_d = [seg[p * NXB // 2 : (p + 1) * NXB // 2]
               .bitcast(I8).rearrange("(t c f) -> t c f", c=C, f=F)
               for p in range(2)]
        sc_d = seg[OFF_SC // 2 : OFF_WS // 2].bitcast(FP).rearrange(
            "(c f) -> c f", f=F)

        # ---- gather weight shards on device, then load ------------------
        wi = dram.tile([1, WSH_H], F16, tag="wi", name="wi")
        wo = dram.tile([1, g * WSH_H], F16, tag="wo", name="wo")
        nc.sync.dma_start(wi[:], go[:, OFF_WS // 2 : SEG_H])
        nc.gpsimd.collective_compute(
            "AllGather", AL.bypass,
            replica_groups=cc_all,
            ins=[wi.opt()], outs=[wo.opt()])
        wb = wo[:].squeeze(0)                        # [g*WSH_H] f16 AP
        w1_d = wb[WB_W1 // 2 : WB_W2 // 2].rearrange("(f u) -> f u", u=U)
        w2_d = wb[WB_W2 // 2 : WB_B1 // 2].rearrange("(u f) -> u f", f=F)
        b1_d = wb[WB_B1 // 2 : WB_B2 // 2].bitcast(FP)
        b2_d = wb[WB_B2 // 2 : WBLOB // 2].bitcast(FP)

        w1s = st.tile([128, NJ * U], F16, tag="w1s", name="w1s")
        w2s = st.tile([128, 2 * F], F16, tag="w2s", name="w2s")
        for j in range(NJ):
            nc.sync.dma_start(w1s[0 : FSZ[j], j * U : (j + 1) * U],
                              w1_d[128 * j : 128 * j + FSZ[j], :])
            nc.sync.dma_start(b2t[0 : FSZ[j], j : j + 1],
                              b2_d[128 * j : 128 * j + FSZ[j]].rearrange("(p o) -> p o", o=1))
        for jc in range(2):
            nc.sync.dma_start(w2s[:, jc * F : (jc + 1) * F],
                              w2_d[128 * jc : 128 * (jc + 1), :])
            nc.sync.dma_start(b1t[:, jc : jc + 1],
                              b1_d[128 * jc : 128 * (jc + 1)].rearrange("(p o) -> p o", o=1))
        nc.scalar.copy(W1t[:], w1s[:])
        nc.scalar.copy(W2t[:], w2s[:])

        # ---- load scales: per-(c,f) dequant factors ---------------------
        for j in range(NJ):
            for c in range(C):
                nc.sync.dma_start(
                    sct[0 : FSZ[j], j * C + c : j * C + c + 1],
                    sc_d[c, 128 * j : 128 * j + FSZ[j]].rearrange("(p o) -> p o", o=1))

        # ---- load input planes: int8 (t,f) tiles -> f16 -> PE transpose
        # -> per-partition dequant scale -> fp32 (f,t) state
        for c in range(C):
            for p in range(2):
                for ti, th in enumerate(TT_SIZES):
                    it8 = scr.tile([128, F], I8, tag="ld8", name="ld8", bufs=2)
                    nc.sync.dma_start(it8[0:th, :],
                                      x_d[p][ti * 128 : ti * 128 + th, c, :])
                    it16 = scr.tile([128, F], F16, tag="ld16", name="ld16", bufs=2)
                    nc.scalar.copy(it16[0:th, :], it8[0:th, :])
                    for j in range(NJ):
                        fj = FSZ[j]
                        ps = psB.tile([128, 128], F16, tag="tp16", name="tp16")
                        nc.tensor.transpose(ps[0:fj, 0:th],
                                            it16[0:th, 128 * j : 128 * j + fj],
                                            id16[0:th, 0:th])
                        nc.scalar.mul(
                            Y[c][p][0:fj, j * TL + ti * 128 : j * TL + ti * 128 + th],
                            ps[0:fj, 0:th], sct[0:fj, j * C + c : j * C + c + 1])
        for p in range(2):
            nc.vector.tensor_copy(X0[p][:], Y[0][p][:])

        # ---- helper groups ---------------------------------------------
        def qs(q):            # [128, NJ] AP of quantity q in S
            return S[:, q * NJ : (q + 1) * NJ]

        def mask_phase():
            for c in range(C):
                ph = [psA.tile([128, TL], FP, tag="ph", name="ph") for _ in range(2)]
                for j in range(NJ):
                    fj = FSZ[j]
                    s1 = scr.tile([128, TL], FP, tag="sq", name="sq", bufs=4)
                    s2 = scr.tile([128, TL], FP, tag="sq", name="sq", bufs=4)
                    nc.scalar.activation(s1[0:fj, :], _fslice(Y[c][0], j, TL), AF.Square)
                    nc.scalar.activation(s2[0:fj, :], _fslice(Y[c][1], j, TL), AF.Square)
                    nc.gpsimd.tensor_add(_fslice(A[c], j, TL), s1[0:fj, :], s2[0:fj, :])
                    ft = featp.tile([128, TL], FP, tag="ft", name="ft", bufs=4)
                    nc.scalar.activation(ft[0:fj, :], _fslice(A[c], j, TL), AF.Ln,
                                         bias=1.0)
                    for m in range(2):
                        nc.tensor.matmul(
                            ph[m][:, :],
                            W1t[0:fj, j * U + 128 * m : j * U + 128 * (m + 1)],
                            ft[0:fj, :],
                            start=(j == 0), stop=(j == NJ - 1))
                ht = hp.tile([128, 2 * TL], FP, tag="ht", name="ht")
                for m in range(2):
                    nc.scalar.activation(ht[:, m * TL : (m + 1) * TL], ph[m][:, :],
                                         AF.Tanh, bias=b1t[:, m : m + 1])
                for j in range(NJ):
                    fj = FSZ[j]
                    pm = psB.tile([128, TL], FP, tag="pm", name="pm")
                    for jc in range(2):
                        nc.tensor.matmul(
                            pm[0:fj, :],
                            W2t[:, jc * F + 128 * j : jc * F + 128 * j + fj],
                            ht[:, jc * TL : (jc + 1) * TL],
                            start=(jc == 0), stop=(jc == 1))
                    nc.scalar.activation(_fslice(Wm[c], j, TL), pm[0:fj, :],
                                         AF.Sigmoid, bias=b2t[0:fj, j : j + 1])

        def stats_phase():
            for j in range(NJ):
                fj = FSZ[j]
                y0r, y0i = _fslice(Y[0][0], j, TL), _fslice(Y[0][1], j, TL)
                y1r, y1i = _fslice(Y[1][0], j, TL), _fslice(Y[1][1], j, TL)
                m1 = scr.tile([128, TL], BF, tag="pp", name="pp", bufs=4)
                m2 = scr.tile([128, TL], BF, tag="pp", name="pp", bufs=4)
                pr = scr.tile([128, TL], BF, tag="pr", name="pr", bufs=2)
                nc.vector.tensor_mul(m1[0:fj, :], y1r, y0r)
                nc.vector.tensor_mul(m2[0:fj, :], y1i, y0i)
                nc.vector.tensor_add(pr[0:fj, :], m1[0:fj, :], m2[0:fj, :])
                m3 = scr.tile([128, TL], BF, tag="pp", name="pp", bufs=4)
                m4 = scr.tile([128, TL], BF, tag="pp", name="pp", bufs=4)
                pi = scr.tile([128, TL], BF, tag="pi", name="pi", bufs=2)
                nc.gpsimd.tensor_mul(m3[0:fj, :], y1i, y0r)
                nc.gpsimd.tensor_mul(m4[0:fj, :], y1r, y0i)
                nc.gpsimd.tensor_sub(pi[0:fj, :], m3[0:fj, :], m4[0:fj, :])
                srcs = [(Wm[0], _fslice(A[0], j, TL), 0),
                        (Wm[1], _fslice(A[0], j, TL), 1),
                        (Wm[0], _fslice(A[1], j, TL), 2),
                        (Wm[1], _fslice(A[1], j, TL), 3),
                        (Wm[0], pr[0:fj, :], 4), (Wm[0], pi[0:fj, :], 5),
                        (Wm[1], pr[0:fj, :], 6), (Wm[1], pi[0:fj, :], 7)]
                for wt, src_ap, q in srcs:
                    prod = scr.tile([128, TL], BF, tag="pd", name="pd", bufs=6)
                    eng = nc.vector if q % 2 == 0 else nc.gpsimd
                    eng.tensor_mul(prod[0:fj, :], _fslice(wt, j, TL), src_ap)
                    nc.vector.tensor_reduce(
                        S[0:fj, q * NJ + j : q * NJ + j + 1], prod[0:fj, :],
                        axis=mybir.AxisListType.X, op=AL.add)

        def allreduce(tile_t, ncols):
            bi = dram.tile([128, ncols], FP, tag="cin", name="cin", bufs=2)
            bo = dram.tile([128, ncols], FP, tag="cout", name="cout", bufs=2)
            nc.sync.dma_start(bi[:], tile_t[:, 0:ncols])
            nc.gpsimd.collective_compute(
                "AllReduce", AL.add,
                replica_groups=cc_pairs,
                ins=[bi.opt()], outs=[bo.opt()])
            nc.sync.dma_start(tile_t[:, 0:ncols], bo[:])

        def smalls():
            """Per-(f) coefficient algebra on [128, NJ] tiles."""
            def t():
                return sm.tile([128, NJ], FP, tag="smt", name="smt")

            def c(name):
                return cf.tile([128, NJ], FP, tag=name, name=name)
            invT = 1.0 / float(T)
            d0, r0 = t(), t()
            alpha = c("alpha")
            nc.vector.tensor_scalar(d0[:], qs(0), invT, EPS, AL.mult, AL.max)
            nc.vector.reciprocal(r0[:], d0[:])
            nc.scalar.activation(alpha[:], r0[:], AF.Sqrt)
            d1, r1 = t(), t()
            nc.vector.tensor_scalar(d1[:], qs(1), EPS, None, AL.max)
            nc.vector.reciprocal(r1[:], d1[:])
            vr = t()
            vi, nvr, nvi = c("vi"), c("nvr"), c("nvi")
            nc.vector.tensor_mul(vr[:], qs(6), r1[:])
            nc.vector.tensor_mul(vi[:], qs(7), r1[:])
            nc.vector.tensor_scalar_mul(nvr[:], vr[:], -1.0)
            nc.vector.tensor_scalar_mul(nvi[:], vi[:], -1.0)
            m2, u = t(), t()
            nc.vector.tensor_mul(m2[:], vr[:], vr[:])
            nc.vector.scalar_tensor_tensor(u[:], vi[:], 1.0, vi[:], AL.mult, AL.mult)
            nc.vector.tensor_add(m2[:], m2[:], u[:])
            # den0' = q2 - 2(vr q4 + vi q5) + m2 q0 ; den1' likewise with q6,q7,q1,q3
            def denp(qa, qb, qden, qs11):
                x1, x2, e = t(), t(), t()
                nc.vector.tensor_mul(x1[:], vr[:], qa)
                nc.vector.scalar_tensor_tensor(x2[:], vi[:], 1.0, qb, AL.mult, AL.mult)
                nc.vector.tensor_add(x1[:], x1[:], x2[:])
                nc.vector.tensor_mul(e[:], m2[:], qden)
                o = t()
                nc.vector.scalar_tensor_tensor(o[:], x1[:], -2.0, qs11, AL.mult, AL.add)
                nc.vector.tensor_add(o[:], o[:], e[:])
                return o
            den0p = denp(qs(4), qs(5), qs(0), qs(2))
            den1p = denp(qs(6), qs(7), qs(1), qs(3))
            dm, rdm = t(), t()
            nc.vector.tensor_scalar(dm[:], den0p[:], EPS, None, AL.max)
            nc.vector.reciprocal(rdm[:], dm[:])
            # v1 = alpha*((q4,-q5) - conj(v) q0) / den0p
            v1r, tA, tB = t(), t(), t()
            v1i, nv1r, nv1i = c("v1i"), c("nv1r"), c("nv1i")
            nc.vector.tensor_mul(tA[:], vr[:], qs(0))
            nc.vector.tensor_sub(tA[:], qs(4), tA[:])
            nc.vector.tensor_mul(tA[:], tA[:], alpha[:])
            nc.vector.tensor_mul(v1r[:], tA[:], rdm[:])
            nc.vector.tensor_mul(tB[:], vi[:], qs(0))
            nc.vector.tensor_sub(tB[:], tB[:], qs(5))
            nc.vector.tensor_mul(tB[:], tB[:], alpha[:])
            nc.vector.tensor_mul(v1i[:], tB[:], rdm[:])
            nc.vector.tensor_scalar_mul(nv1r[:], v1r[:], -1.0)
            nc.vector.tensor_scalar_mul(nv1i[:], v1i[:], -1.0)
            db, rb = t(), t()
            beta = c("beta")
            nc.vector.tensor_scalar(db[:], den1p[:], invT, EPS, AL.mult, AL.max)
            nc.vector.reciprocal(rb[:], db[:])
            nc.scalar.activation(beta[:], rb[:], AF.Sqrt)
            return alpha, beta, vi, nvr, nvi, v1i, nv1r, nv1i

        def apply_phase(alpha, beta, vi, nvr, nvi, v1i, nv1r, nv1i):
            for j in range(NJ):
                fj = FSZ[j]
                y0r, y0i = _fslice(Y[0][0], j, TL), _fslice(Y[0][1], j, TL)
                y1r, y1i = _fslice(Y[1][0], j, TL), _fslice(Y[1][1], j, TL)
                def c_(ct):
                    return ct[0:fj, j : j + 1]
                t1 = scr.tile([128, TL], FP, tag="ap", name="ap", bufs=4)
                y1pr = scr.tile([128, TL], FP, tag="y1p", name="y1p")
                nc.vector.scalar_tensor_tensor(t1[0:fj, :], y0r, c_(nvr), y1r,
                                               AL.mult, AL.add)
                nc.vector.scalar_tensor_tensor(y1pr[0:fj, :], y0i, c_(vi), t1[0:fj, :],
                                               AL.mult, AL.add)
                t2 = scr.tile([128, TL], FP, tag="ap", name="ap", bufs=4)
                y1pi = scr.tile([128, TL], FP, tag="y1p", name="y1p")
                nc.vector.scalar_tensor_tensor(t2[0:fj, :], y0i, c_(nvr), y1i,
                                               AL.mult, AL.add)
                nc.vector.scalar_tensor_tensor(y1pi[0:fj, :], y0r, c_(nvi), t2[0:fj, :],
                                               AL.mult, AL.add)
                s1 = scr.tile([128, TL], FP, tag="ap", name="ap", bufs=4)
                s2 = scr.tile([128, TL], FP, tag="ap", name="ap", bufs=4)
                nc.scalar.mul(s1[0:fj, :], y0r, c_(alpha))
                nc.scalar.mul(s2[0:fj, :], y0i, c_(alpha))
                t3 = scr.tile([128, TL], FP, tag="ap", name="ap", bufs=4)
                nc.vector.scalar_tensor_tensor(t3[0:fj, :], y1pr[0:fj, :], c_(nv1r),
                                               s1[0:fj, :], AL.mult, AL.add)
                nc.vector.scalar_tensor_tensor(y0r, y1pi[0:fj, :], c_(v1i),
                                               t3[0:fj, :], AL.mult, AL.add)
                t4 = scr.tile([128, TL], FP, tag="ap", name="ap", bufs=4)
                nc.vector.scalar_tensor_tensor(t4[0:fj, :], y1pi[0:fj, :], c_(nv1r),
                                               s2[0:fj, :], AL.mult, AL.add)
                nc.vector.scalar_tensor_tensor(y0i, y1pr[0:fj, :], c_(nv1i),
                                               t4[0:fj, :], AL.mult, AL.add)
                nc.scalar.mul(y1r, y1pr[0:fj, :], c_(beta))
                nc.scalar.mul(y1i, y1pi[0:fj, :], c_(beta))

        def mt():
            return sm.tile([128, NJ], FP, tag="mup", name="mup")

        def m_update(alpha, beta, vi, nvr, nvi, v1i, nv1r, nv1i):
            """Accumulate the iteration's 2x2 steering matrix into M.

            Same algebra as apply_phase, applied to each column k of M
            (elementwise on [128, NJ] tiles: one coef per (f, j))."""
            for k in range(2):
                m0r, m0i = Mv(k, 0, 0), Mv(k, 0, 1)
                m1r, m1i = Mv(k, 1, 0), Mv(k, 1, 1)
                t1, t2 = mt(), mt()
                y1pr, y1pi = mt(), mt()
                nc.vector.tensor_mul(t1[:], nvr[:], m0r)
                nc.vector.tensor_add(t1[:], t1[:], m1r)
                nc.vector.tensor_mul(t2[:], vi[:], m0i)
                nc.vector.tensor_add(y1pr[:], t1[:], t2[:])
                t3, t4 = mt(), mt()
                nc.vector.tensor_mul(t3[:], nvr[:], m0i)
                nc.vector.tensor_add(t3[:], t3[:], m1i)
                nc.vector.tensor_mul(t4[:], nvi[:], m0r)
                nc.vector.tensor_add(y1pi[:], t3[:], t4[:])
                s1, u1, u2 = mt(), mt(), mt()
                nc.vector.tensor_mul(s1[:], alpha[:], m0r)
                nc.vector.tensor_mul(u1[:], nv1r[:], y1pr[:])
                nc.vector.tensor_add(s1[:], s1[:], u1[:])
                nc.vector.tensor_mul(u2[:], v1i[:], y1pi[:])
                s2, u3, u4 = mt(), mt(), mt()
                nc.vector.tensor_mul(s2[:], alpha[:], m0i)
                nc.vector.tensor_mul(u3[:], nv1r[:], y1pi[:])
                nc.vector.tensor_add(s2[:], s2[:], u3[:])
                nc.vector.tensor_mul(u4[:], nv1i[:], y1pr[:])
                nc.vector.tensor_add(m0r, s1[:], u2[:])
                nc.vector.tensor_add(m0i, s2[:], u4[:])
                nc.vector.tensor_mul(m1r, beta[:], y1pr[:])
                nc.vector.tensor_mul(m1i, beta[:], y1pi[:])

        # ---- main loop ---------------------------------------------------
        n_it = int(os.environ.get("KITERS", str(N_ITER)))
        do_cc = os.environ.get("KCC", "1") == "1"
        do_pb = os.environ.get("KPB", "1") == "1"
        do_mask = os.environ.get("KMASK", "1") == "1"
        do_stats = os.environ.get("KSTATS", "1") == "1"
        do_apply = os.environ.get("KAPPLY", "1") == "1"
        for _ in range(n_it):
            if do_mask:
                mask_phase()
            if do_stats:
                stats_phase()
            if do_cc:
                allreduce(S, 8 * NJ)
            if do_apply:
                coefs = smalls()
                apply_phase(*coefs)
                m_update(*coefs)

        # ---- projection back: stats over final Y, fold scale into M -----
        for j in ([] if not do_pb else range(NJ)):
            fj = FSZ[j]
            for c in range(C):
                pairs = [(Y[c][0], X0[0]), (Y[c][1], X0[1]),
                         (Y[c][0], X0[1]), (Y[c][1], X0[0]),
                         (Y[c][0], Y[c][0]), (Y[c][1], Y[c][1])]
                for qi, (ta, tb) in enumerate(pairs):
                    q = c * 6 + qi
                    prod = scr.tile([128, TL], FP, tag="pd2", name="pd2", bufs=4)
                    if qi >= 4:
                        nc.scalar.activation(prod[0:fj, :], _fslice(ta, j, TL),
                                             AF.Square)
                    else:
                        eng = nc.vector if qi % 2 == 0 else nc.gpsimd
                        eng.tensor_mul(prod[0:fj, :], _fslice(ta, j, TL),
                                       _fslice(tb, j, TL))
                    nc.vector.tensor_reduce(
                        PB[0:fj, q * NJ + j : q * NJ + j + 1], prod[0:fj, :],
                        axis=mybir.AxisListType.X, op=AL.add)
        if do_pb:
            allreduce(PB, 12 * NJ)

        def pbq(q):
            return PB[:, q * NJ : (q + 1) * NJ]

        for c in ([] if not do_pb else range(C)):
            gq = [pbq(c * 6 + i) for i in range(6)]
            numr = sm.tile([128, NJ], FP, tag="pbs", name="pbs")
            numi = sm.tile([128, NJ], FP, tag="pbs", name="pbs")
            den = sm.tile([128, NJ], FP, tag="pbs", name="pbs")
            rc = sm.tile([128, NJ], FP, tag="pbs", name="pbs")
            cr = sm.tile([128, NJ], FP, tag=f"cr{c}", name=f"cr{c}")
            ci = sm.tile([128, NJ], FP, tag=f"ci{c}", name=f"ci{c}")
            nc.vector.tensor_add(numr[:], gq[0], gq[1])
            nc.vector.tensor_sub(numi[:], gq[2], gq[3])
            nc.vector.tensor_add(den[:], gq[4], gq[5])
            nc.vector.tensor_scalar(den[:], den[:], EPS, None, AL.max)
            nc.vector.reciprocal(rc[:], den[:])
            nc.vector.tensor_mul(cr[:], numr[:], rc[:])
            nc.vector.tensor_mul(ci[:], numi[:], rc[:])
            # M[c, :] *= (cr + i ci)  for both columns k
            for k in range(2):
                mr, mi = Mv(k, c, 0), Mv(k, c, 1)
                u = mt()
                w = mt()
                minew = mt()
                nc.vector.tensor_mul(u[:], cr[:], mi)
                nc.vector.tensor_mul(w[:], ci[:], mr)
                nc.vector.tensor_add(minew[:], u[:], w[:])
                u2, w2 = mt(), mt()
                nc.vector.tensor_mul(u2[:], cr[:], mr)
                nc.vector.tensor_mul(w2[:], ci[:], mi)
                nc.vector.tensor_sub(mr, u2[:], w2[:])
                nc.vector.tensor_copy(mi, minew[:])

        # ---- ship M: gather all cores' M blocks, DMA to output ----------
        mo_i = dram.tile([1, 128 * 8 * NJ], FP, tag="moi", name="moi")
        mo_o = dram.tile([1, OLEN_M], FP, tag="moo", name="moo")
        nc.sync.dma_start(
            mo_i[:].squeeze(0).rearrange("(p k) -> p k", k=8 * NJ), Mt[:])
        nc.gpsimd.collective_compute(
            "AllGather", AL.bypass,
            replica_groups=cc_all,
            ins=[mo_i.opt()], outs=[mo_o.opt()])
        nc.sync.dma_start(po_d.rearrange("(o k) -> o k", o=1), mo_o[:])


class _Exec:
    """Cached jitted shard_map executor: one put to core 0, zero-shard
    dummies for cores 1-7, tiny single-shard M fetch."""

    def __init__(self, nc, devices=None, g=N_CORES):
        import jax
        from jax.sharding import Mesh, PartitionSpec, NamedSharding
        from jax.experimental.shard_map import shard_map
        from concourse.bass2jax import (
            _bass_exec_p, install_neuronx_cc_hook, partition_id_tensor,
        )
        import jax.numpy as jnp

        self.jax = jax
        install_neuronx_cc_hook()
        partition_name = (nc.partition_id_tensor.name
                          if nc.partition_id_tensor else None)
        in_names, out_names, out_avals = [], [], []
        in_len = None
        for alloc in nc.m.functions[0].allocations:
            if not isinstance(alloc, mybir.MemoryLocationSet):
                continue
            name = alloc.memorylocations[0].name
            if alloc.kind == "ExternalInput":
                if name != partition_name:
                    in_names.append(name)
                    in_len = int(alloc.tensor_shape[0])
            elif alloc.kind == "ExternalOutput":
                out_names.append(name)
                out_avals.append(jax.core.ShapedArray(
                    tuple(alloc.tensor_shape), mybir.dt.np(alloc.dtype)))
        assert in_names == ["pk"] and out_names == ["po"], (in_names, out_names)
        assert in_len == PK_H, in_len
        n_params = len(in_names)
        n_outs = len(out_avals)
        all_in = list(in_names) + list(out_names)
        if partition_name is not None:
            all_in.append(partition_name)

        def _bdy(*args):
            operands = list(args)
            if partition_name is not None:
                operands.append(partition_id_tensor())
            return tuple(_bass_exec_p.bind(
                *operands,
                out_avals=tuple(out_avals),
                in_names=tuple(all_in),
                out_names=tuple(out_names),
                lowering_input_output_aliases=(),
                sim_require_finite=True,
                sim_require_nnan=True,
                nc=nc,
            ))

        if devices is None:
            devices = jax.devices()[:g]
        assert len(devices) == g
        self.devices = devices
        self.g = g
        mesh = Mesh(np.asarray(devices), ("core",))
        self.shard = NamedSharding(mesh, PartitionSpec("core"))
        in_specs = (PartitionSpec("core"),) * (n_params + n_outs)
        out_specs = (PartitionSpec("core"),) * n_outs
        jitfn = jax.jit(
            shard_map(_bdy, mesh=mesh, in_specs=in_specs, out_specs=out_specs,
                      check_rep=False),
            donate_argnums=tuple(range(n_params, n_params + n_outs)),
            keep_unused=True,
        )
        aval = out_avals[0]
        gshape = (g * aval.shape[0],)
        gdtype = aval.dtype
        try:
            from concourse.bass2jax import fast_dispatch_compile
            pk_spec = jax.ShapeDtypeStruct((g * PK_H,), np.float16,
                                           sharding=self.shard)
            do_spec = jax.ShapeDtypeStruct(gshape, gdtype, sharding=self.shard)
            self.sharded = fast_dispatch_compile(
                lambda: jitfn.lower(pk_spec, do_spec).compile())
        except Exception:
            self.sharded = jitfn
        self.mkzeros = jax.jit(lambda: jnp.zeros(gshape, gdtype),
                               out_shardings=self.shard)
        # persistent zero input shards for cores 1..7 (never transferred)
        zin = jax.jit(lambda: jnp.zeros((g * PK_H,), jnp.float16),
                      out_shardings=self.shard)()
        zin.block_until_ready()
        self.zsh = [s.data for s in zin.addressable_shards]
        self._zin = zin
        self.prev_out = None
        self.pack_buf = np.empty(PK_B, np.int8)

    def run(self, inputs):
        """Pack + quantize, single device_put to core 0, dispatch, fetch M."""
        jax = self.jax
        donate = self.prev_out if self.prev_out is not None else self.mkzeros()
        dr = np.asarray(inputs["data_real"])
        di = np.asarray(inputs["data_imag"])
        buf = self.pack_buf
        wblob = np.zeros(N_CORES * WSHB, np.uint8)
        wblob[WB_W1 : WB_W2] = np.ascontiguousarray(
            np.asarray(inputs["W1"], np.float32).astype(np.float16)).view(np.uint8).ravel()
        wblob[WB_W2 : WB_B1] = np.ascontiguousarray(
            np.asarray(inputs["W2"], np.float32).astype(np.float16)).view(np.uint8).ravel()
        wblob[WB_B1 : WB_B2] = np.ascontiguousarray(
            np.asarray(inputs["b1"], np.float32)).view(np.uint8).ravel()
        wblob[WB_B2 : WBLOB] = np.ascontiguousarray(
            np.asarray(inputs["b2"], np.float32)).view(np.uint8).ravel()
        wv = wblob.view(np.int8)

        tmp = self.__dict__.setdefault(
            "qtmp", np.empty((TL, C, F), np.float32))
        q8 = self.__dict__.setdefault(
            "q8tmp", np.empty((TL, C, F), np.int8))
        ab = self.__dict__.setdefault("abtmp", np.empty((T, C, F), np.float32))
        for b in range(B):
            np.abs(dr[b], out=ab)
            a = ab.max(axis=0)
            np.abs(di[b], out=ab)
            np.maximum(a, ab.max(axis=0), out=a)
            np.maximum(a, 1e-20, out=a)
            inv = np.float32(QLVL) / a                      # (C,F)
            sc = (a * np.float32(1.0 / QLVL)).astype(np.float32)
            scv = sc.ravel().view(np.int8)
            for h in range(TSPLIT):
                k = TSPLIT * b + h
                seg = buf[k * SEG_B : (k + 1) * SEG_B]
                sl = slice(h * TL, (h + 1) * TL)
                for p, src in ((0, dr), (1, di)):
                    np.multiply(src[b, sl], inv, out=tmp)
                    np.rint(tmp, out=tmp)
                    np.copyto(q8, tmp, casting="unsafe")
                    seg[p * NXB : (p + 1) * NXB] = q8.ravel()
                seg[OFF_SC : OFF_WS] = scv
                seg[OFF_WS : SEG_B] = wv[k * WSHB : (k + 1) * WSHB]

        s0 = jax.device_put(buf.view(np.float16), self.devices[0])
        arr = jax.make_array_from_single_device_arrays(
            (self.g * PK_H,), self.shard, [s0] + self.zsh[1:])
        (out,) = self.sharded(arr, donate)
        sh0 = out.addressable_shards[0].data
        try:
            sh0.copy_to_host_async()
        except Exception:
            pass
        res = np.asarray(sh0)
        self.prev_out = out
        return res, dr, di


def _unpack_apply(res, dr, di):
    """Decode per-b 2x2 demixing matrices and apply them to the full-
    precision input on the host: out[s,b,t,f] = sum_k M[b,s,k,f] X_k."""
    blocks = res.reshape(N_CORES, 128, 8 * NJ)
    Mk = np.empty((B, 2, 2, 2, F), np.float32)      # [b, k, c, p, f]
    for b in range(B):
        blk = blocks[2 * b]
        mb = Mk[b].reshape(8, F)
        for j in range(NJ):
            fj = FSZ[j]
            mb[:, 128 * j : 128 * j + fj] = blk[0:fj, j::NJ].T
    out = np.empty((2, B, T, F), np.complex64)
    our, oui = out.real, out.imag
    tmp = np.empty((T, F), np.float32)
    for b in range(B):
        xr0, xi0 = dr[b, :, 0, :], di[b, :, 0, :]
        xr1, xi1 = dr[b, :, 1, :], di[b, :, 1, :]
        for s in range(2):
            ar, ai = Mk[b, 0, s, 0][None, :], Mk[b, 0, s, 1][None, :]
            br, bi = Mk[b, 1, s, 0][None, :], Mk[b, 1, s, 1][None, :]
            acc = our[s, b]
            np.multiply(xr0, ar, out=acc)
            np.multiply(xi0, ai, out=tmp)
            np.subtract(acc, tmp, out=acc)
            np.multiply(xr1, br, out=tmp)
            np.add(acc, tmp, out=acc)
            np.multiply(xi1, bi, out=tmp)
            np.subtract(acc, tmp, out=acc)
            acc = oui[s, b]
            np.multiply(xi0, ar, out=acc)
            np.multiply(xr0, ai, out=tmp)
            np.add(acc, tmp, out=acc)
            np.multiply(xi1, br, out=tmp)
            np.add(acc, tmp, out=acc)
            np.multiply(xr1, bi, out=tmp)
            np.add(acc, tmp, out=acc)
    return out


def kernel(**inputs):
    if "ex" not in _CACHED:
        _CACHED["nc"] = _build()
        _CACHED["ex"] = _Exec(_CACHED["nc"])
    ex = _CACHED["ex"]
    res, dr, di = ex.run(inputs)
    return _unpack_apply(res, dr, di)


if __name__ == "__main__":
    rng = np.random.default_rng(0)
    ins = {
        "data_real": rng.standard_normal((B, T, C, F), dtype=np.float32),
        "data_imag": rng.standard_normal((B, T, C, F), dtype=np.float32),
        "ilens": np.full((B,), T, dtype=np.int32),
        "W1": rng.standard_normal((F, U), dtype=np.float32) / np.sqrt(F),
        "b1": np.zeros((U,), dtype=np.float32),
        "W2": rng.standard_normal((U, F), dtype=np.float32) / np.sqrt(U),
        "b2": np.zeros((F,), dtype=np.float32),
    }
    out = kernel(**ins)
    print("kernel ran", out.shape, out.dtype, np.abs(out).mean())


# revision 7
# speedup vs baseline: 1.5613x; 1.2095x over previous
"""Trainium2 Bass kernel for DNN-IVA (15-iteration ISS + per-frame MLP mask net).

v2: demixing-matrix output + single-put broadcast input.

Key reformulation: every ISS source-step is LINEAR in the spectrogram X
(Y <- A Y with a per-(b,f) 2x2 complex A; projection-back is a diagonal
scale).  So the final output is  out = M(b,f) . X  with M a tiny per-(b,f)
2x2 complex matrix.  The device accumulates M alongside Y (a few dozen
[128,NJ]-sized vector ops per iteration) and ships ONLY M (160 KB) back;
the host (which already holds X at full precision) applies M in ~40 ms.
This removes the 8 MB quantized output fetch entirely AND its int8 error.

Host->device traffic is ONE ~8.8 MB int8 device_put to core 0: the packed
buffer holds all 8 cores' per-core segments (int8-quantized X slices +
fp32 scales + an 1/8 weight shard each).  Cores 1-7 receive persistent
on-device zero buffers (no transfer).  In-kernel, a ReduceScatter(add)
over all 8 cores delivers segment k to core k (zeros + core0's data =
core0's data), then an AllGather reassembles the full mask-net weights
from the 8 shards.  The axon tunnel moves 8.8 MB in + 160 KB out per call
instead of the 17 MB + 8.2 MB of the per-core-put scheme.

Compute sharding: data-parallel over B (4) x T (2) = 8 cores, per-iteration
ISS statistics pair-AllReduced over the T halves (20 KB), as before.
On-chip layout: f on partitions (5 chunks of 128), t on the free dim.
"""

import os

import numpy as np

import concourse.bass as bass
import concourse.tile as tile
from concourse import bacc, mybir, masks

B, T, C, F, U = 4, 1000, 2, 513, 256
N_ITER = 15
EPS = 1e-6
N_CORES = 8
TSPLIT = 2
TL = T // TSPLIT          # 500 local frames per core
NJ = 5                    # f chunks of 128 (last has 1 valid row)
FSZ = [128, 128, 128, 128, 1]
TT_SIZES = [128, 128, 128, 116]   # t tiles covering TL=500 for load/store
FP = mybir.dt.float32
F16 = mybir.dt.float16
BF = mybir.dt.bfloat16
I8 = mybir.dt.int8
I32 = mybir.dt.int32
AL = mybir.AluOpType
AF = mybir.ActivationFunctionType

# ---- packed int8 input layout --------------------------------------------
# per-core segment: [X re int8 | X im int8 | scales f32 | weight shard]
NXB = TL * C * F                  # 513000 bytes per plane
OFF_SC = 2 * NXB                  # 1026000
SCB = C * F * 4                   # 4104 bytes of fp32 scales
OFF_WS = OFF_SC + SCB             # 1030104
# weight blob: W1 f16 | W2 f16 | b1 f32 | b2 f32 (byte offsets)
NW = F * U                        # 131328 elems
WB_W1 = 0
WB_W2 = WB_W1 + 2 * NW            # 262656
WB_B1 = WB_W2 + 2 * NW            # 525312
WB_B2 = WB_B1 + 4 * U             # 526336
WBLOB = WB_B2 + 4 * F             # 528388 bytes
WSHB = ((WBLOB + N_CORES - 1) // N_CORES + 3) // 4 * 4   # 66052 per core
SEG_B = OFF_WS + WSHB             # 1096156 bytes per core
SEG_H = SEG_B // 2                # 548078 f16 units
PK_B = N_CORES * SEG_B            # 8769248
PK_H = PK_B // 2                  # 4384624 f16 units per-core external input
WSH_H = WSHB // 2                 # 33026
OLEN_M = N_CORES * 128 * 8 * NJ   # 40960 fp32 output (all cores' M blocks)
# Quantization level: values in [-QLVL, QLVL].  Coarser than int8's 127 on
# purpose: the tunnel compresses, so lower payload entropy = fewer wire
# bytes, while rel-err stays ~9e-3 (tolerance 2e-2).  Device side is
# unchanged (plain int8 bytes + shipped scales).
QLVL = 45.0

_CACHED = {}

# ---- optional C fast path for host pack/apply (numpy fallback) -----------
_CSRC = r"""
#include <math.h>

void apply_m(const float* restrict dr, const float* restrict di,
             const float* restrict mk, float* restrict out,
             int B, int T, int C, int F) {
    for (int b = 0; b < B; b++) {
        const float* m = mk + (long)b*8*F;
        for (int t = 0; t < T; t++) {
            const float* xr0 = dr + (((long)b*T + t)*C + 0)*F;
            const float* xr1 = xr0 + F;
            const float* xi0 = di + (((long)b*T + t)*C + 0)*F;
            const float* xi1 = xi0 + F;
            for (int s = 0; s < 2; s++) {
                const float* ar = m + (0*2 + s)*2*F;
                const float* ai = ar + F;
                const float* br = m + (4 + 2*s)*F;
                const float* bi = br + F;
                float* o = out + ((((long)s*B + b)*T + t)*F)*2;
                for (int f = 0; f < F; f++) {
                    o[2*f]   = ar[f]*xr0[f] - ai[f]*xi0[f]
                             + br[f]*xr1[f] - bi[f]*xi1[f];
                    o[2*f+1] = ar[f]*xi0[f] + ai[f]*xr0[f]
                             + br[f]*xi1[f] + bi[f]*xr1[f];
                }
            }
        }
    }
}

void quant_pack(const float* restrict dr, const float* restrict di,
                signed char* restrict buf, float* restrict scout,
                float qlvl, int B, int T, int C, int F,
                long seg_b, long nxb, long off_sc) {
    int TL = T/2;
    int CF = C*F;
    for (int b = 0; b < B; b++) {
        const float* drb = dr + (long)b*T*CF;
        const float* dib = di + (long)b*T*CF;
        float* amax = scout + (long)b*CF;
        for (int j = 0; j < CF; j++) amax[j] = 1e-20f;
        for (int t = 0; t < T; t++) {
            const float* r = drb + (long)t*CF;
            const float* i = dib + (long)t*CF;
            for (int j = 0; j < CF; j++) {
                float a = fabsf(r[j]);
                float c = fabsf(i[j]);
                if (c > a) a = c;
                if (a > amax[j]) amax[j] = a;
            }
        }
        for (int h = 0; h < 2; h++) {
            int k = 2*b + h;
            signed char* seg = buf + (long)k*seg_b;
            for (int p = 0; p < 2; p++) {
                const float* src = (p == 0 ? drb : dib) + (long)h*TL*CF;
                signed char* dst = seg + (long)p*nxb;
                for (int t = 0; t < TL; t++) {
                    const float* s = src + (long)t*CF;
                    signed char* d = dst + (long)t*CF;
                    for (int j = 0; j < CF; j++)
                        d[j] = (signed char)lrintf(s[j] * (qlvl / amax[j]));
                }
            }
            float* sc = (float*)(seg + off_sc);
            for (int j = 0; j < CF; j++) sc[j] = amax[j] / qlvl;
        }
    }
}
"""


def _build_chelper():
    """Compile the C fast path; returns ctypes lib or None."""
    import ctypes
    import subprocess
    import tempfile
    try:
        d = tempfile.mkdtemp(prefix="kiva")
        src = os.path.join(d, "h.c")
        so = os.path.join(d, "h.so")
        with open(src, "w") as f:
            f.write(_CSRC)
        subprocess.run(
            ["gcc", "-O3", "-march=native", "-ffast-math", "-shared",
             "-fPIC", "-o", so, src],
            check=True, capture_output=True, timeout=120)
        return ctypes.CDLL(so)
    except Exception:
        return None


def _fslice(tile_ap, j, cols):
    """AP for f-chunk j of a [128, NJ*TL]-shaped plane (cols=TL), valid lanes only."""
    return tile_ap[0 : FSZ[j], j * cols : (j + 1) * cols]


def _build(g=N_CORES):
    nc = bacc.Bacc("TRN2", target_bir_lowering=False, debug=False,
                   num_devices=g)
    pk_d = nc.dram_tensor("pk", [PK_H], F16, kind="ExternalInput").ap()
    po_d = nc.dram_tensor("po", [OLEN_M], FP, kind="ExternalOutput").ap()
    with tile.TileContext(nc) as tc:
        _body(nc, tc, pk_d, po_d, g)
    nc.compile()
    return nc


def _body(nc, tc, pk_d, po_d, g=N_CORES):
    cc_pairs = [[2 * i, 2 * i + 1] for i in range(g // 2)]
    cc_all = [list(range(g))]
    PLANE = NJ * TL
    with (
        tc.tile_pool(name="state", bufs=1) as st,
        tc.tile_pool(name="scr", bufs=3) as scr,
        tc.tile_pool(name="feat", bufs=3) as featp,
        tc.tile_pool(name="hpool", bufs=2) as hp,
        tc.tile_pool(name="small", bufs=12) as sm,
        tc.tile_pool(name="coef", bufs=2) as cf,
        tc.tile_pool(name="psA", bufs=2, space="PSUM") as psA,
        tc.tile_pool(name="psB", bufs=2, space="PSUM") as psB,
        tc.tile_pool(name="dram", bufs=1, space="DRAM") as dram,
    ):
        # ---- persistent state -------------------------------------------
        Y = [[st.tile([128, PLANE], FP, tag=f"Y{c}{p}", name=f"Y{c}{p}") for p in range(2)]
             for c in range(C)]                       # [c][0]=re, [1]=im
        X0 = [st.tile([128, PLANE], FP, tag=f"X0{p}", name=f"X0{p}") for p in range(2)]
        A = [st.tile([128, PLANE], BF, tag=f"a{c}", name=f"a{c}") for c in range(C)]
        Wm = [st.tile([128, PLANE], BF, tag=f"w{c}", name=f"w{c}") for c in range(C)]
        W1t = st.tile([128, NJ * U], FP, tag="W1t", name="W1t")
        W2t = st.tile([128, 2 * F], FP, tag="W2t", name="W2t")
        b1t = st.tile([128, 2], FP, tag="b1t", name="b1t")
        b2t = st.tile([128, NJ], FP, tag="b2t", name="b2t")
        sct = st.tile([128, NJ * C], FP, tag="sct", name="sct")
        Mt = st.tile([128, 8 * NJ], FP, tag="Mt", name="Mt")
        id16 = st.tile([128, 128], F16, tag="id16", name="id16")
        ident = st.tile([128, 128], FP, tag="ident", name="ident")
        S = st.tile([128, 8 * NJ], FP, tag="S", name="S")       # quantity-major
        PB = st.tile([128, 12 * NJ], FP, tag="PB", name="PB")    # projection-back stats

        masks.make_identity(nc, ident[:])
        nc.scalar.copy(id16[:], ident[:])

        def Mv(k, c, p):
            q = (k * 2 + c) * 2 + p
            return Mt[:, q * NJ : (q + 1) * NJ]

        nc.vector.memset(Mt[:], 0.0)
        nc.vector.memset(Mv(0, 0, 0), 1.0)
        nc.vector.memset(Mv(1, 1, 0), 1.0)

        # ---- scatter the single-put packed input to all cores -----------
        # AllToAll (pure bypass, bit-exact): core 0's input holds all 8
        # per-core segments, cores 1-7 hold zeros; core k's output slot 0
        # is core 0's segment k == its own data.  (Reduce-type collectives
        # run their ALU in fp32 here and corrupt raw byte payloads.)
        gi = dram.tile([1, PK_H], F16, tag="gi", name="gi")
        go = dram.tile([1, PK_H], F16, tag="go", name="go")
        nc.sync.dma_start(gi[:], pk_d.rearrange("(o k) -> o k", o=1))
        nc.gpsimd.collective_compute(
            "AllToAll", AL.bypass,
            replica_groups=cc_all,
            ins=[gi.opt()], outs=[go.opt()])
        seg = go[:].squeeze(0)[0:SEG_H]              # [SEG_H] f16 AP
        x_d = [seg[p * NXB // 2 : (p + 1) * NXB // 2]
               .bitcast(I8).rearrange("(t c f) -> t c f", c=C, f=F)
               for p in range(2)]
        sc_d = seg[OFF_SC // 2 : OFF_WS // 2].bitcast(FP).rearrange(
            "(c f) -> c f", f=F)

        # ---- gather weight shards on device, then load ------------------
        wi = dram.tile([1, WSH_H], F16, tag="wi", name="wi")
        wo = dram.tile([1, g * WSH_H], F16, tag="wo", name="wo")
        nc.sync.dma_start(wi[:], go[:, OFF_WS // 2 : SEG_H])
        nc.gpsimd.collective_compute(
            "AllGather", AL.bypass,
            replica_groups=cc_all,
            ins=[wi.opt()], outs=[wo.opt()])
        wb = wo[:].squeeze(0)                        # [g*WSH_H] f16 AP
        w1_d = wb[WB_W1 // 2 : WB_W2 // 2].rearrange("(f u) -> f u", u=U)
        w2_d = wb[WB_W2 // 2 : WB_B1 // 2].rearrange("(u f) -> u f", f=F)
        b1_d = wb[WB_B1 // 2 : WB_B2 // 2].bitcast(FP)
        b2_d = wb[WB_B2 // 2 : WBLOB // 2].bitcast(FP)

        w1s = st.tile([128, NJ * U], F16, tag="w1s", name="w1s")
        w2s = st.tile([128, 2 * F], F16, tag="w2s", name="w2s")
        for j in range(NJ):
            nc.sync.dma_start(w1s[0 : FSZ[j], j * U : (j + 1) * U],
                              w1_d[128 * j : 128 * j + FSZ[j], :])
            nc.sync.dma_start(b2t[0 : FSZ[j], j : j + 1],
                              b2_d[128 * j : 128 * j + FSZ[j]].rearrange("(p o) -> p o", o=1))
        for jc in range(2):
            nc.sync.dma_start(w2s[:, jc * F : (jc + 1) * F],
                              w2_d[128 * jc : 128 * (jc + 1), :])
            nc.sync.dma_start(b1t[:, jc : jc + 1],
                              b1_d[128 * jc : 128 * (jc + 1)].rearrange("(p o) -> p o", o=1))
        nc.scalar.copy(W1t[:], w1s[:])
        nc.scalar.copy(W2t[:], w2s[:])

        # ---- load scales: per-(c,f) dequant factors ---------------------
        for j in range(NJ):
            for c in range(C):
                nc.sync.dma_start(
                    sct[0 : FSZ[j], j * C + c : j * C + c + 1],
                    sc_d[c, 128 * j : 128 * j + FSZ[j]].rearrange("(p o) -> p o", o=1))

        # ---- load input planes: int8 (t,f) tiles -> f16 -> PE transpose
        # -> per-partition dequant scale -> fp32 (f,t) state
        for c in range(C):
            for p in range(2):
                for ti, th in enumerate(TT_SIZES):
                    it8 = scr.tile([128, F], I8, tag="ld8", name="ld8", bufs=2)
                    nc.sync.dma_start(it8[0:th, :],
                                      x_d[p][ti * 128 : ti * 128 + th, c, :])
                    it16 = scr.tile([128, F], F16, tag="ld16", name="ld16", bufs=2)
                    nc.scalar.copy(it16[0:th, :], it8[0:th, :])
                    for j in range(NJ):
                        fj = FSZ[j]
                        ps = psB.tile([128, 128], F16, tag="tp16", name="tp16")
                        nc.tensor.transpose(ps[0:fj, 0:th],
                                            it16[0:th, 128 * j : 128 * j + fj],
                                            id16[0:th, 0:th])
                        nc.scalar.mul(
                            Y[c][p][0:fj, j * TL + ti * 128 : j * TL + ti * 128 + th],
                            ps[0:fj, 0:th], sct[0:fj, j * C + c : j * C + c + 1])
        for p in range(2):
            nc.vector.tensor_copy(X0[p][:], Y[0][p][:])

        # ---- helper groups ---------------------------------------------
        def qs(q):            # [128, NJ] AP of quantity q in S
            return S[:, q * NJ : (q + 1) * NJ]

        def mask_phase():
            for c in range(C):
                ph = [psA.tile([128, TL], FP, tag="ph", name="ph") for _ in range(2)]
                for j in range(NJ):
                    fj = FSZ[j]
                    s1 = scr.tile([128, TL], FP, tag="sq", name="sq", bufs=4)
                    s2 = scr.tile([128, TL], FP, tag="sq", name="sq", bufs=4)
                    nc.scalar.activation(s1[0:fj, :], _fslice(Y[c][0], j, TL), AF.Square)
                    nc.scalar.activation(s2[0:fj, :], _fslice(Y[c][1], j, TL), AF.Square)
                    nc.gpsimd.tensor_add(_fslice(A[c], j, TL), s1[0:fj, :], s2[0:fj, :])
                    ft = featp.tile([128, TL], FP, tag="ft", name="ft", bufs=4)
                    nc.scalar.activation(ft[0:fj, :], _fslice(A[c], j, TL), AF.Ln,
                                         bias=1.0)
                    for m in range(2):
                        nc.tensor.matmul(
                            ph[m][:, :],
                            W1t[0:fj, j * U + 128 * m : j * U + 128 * (m + 1)],
                            ft[0:fj, :],
                            start=(j == 0), stop=(j == NJ - 1))
                ht = hp.tile([128, 2 * TL], FP, tag="ht", name="ht")
                for m in range(2):
                    nc.scalar.activation(ht[:, m * TL : (m + 1) * TL], ph[m][:, :],
                                         AF.Tanh, bias=b1t[:, m : m + 1])
                for j in range(NJ):
                    fj = FSZ[j]
                    pm = psB.tile([128, TL], FP, tag="pm", name="pm")
                    for jc in range(2):
                        nc.tensor.matmul(
                            pm[0:fj, :],
                            W2t[:, jc * F + 128 * j : jc * F + 128 * j + fj],
                            ht[:, jc * TL : (jc + 1) * TL],
                            start=(jc == 0), stop=(jc == 1))
                    nc.scalar.activation(_fslice(Wm[c], j, TL), pm[0:fj, :],
                                         AF.Sigmoid, bias=b2t[0:fj, j : j + 1])

        def stats_phase():
            for j in range(NJ):
                fj = FSZ[j]
                y0r, y0i = _fslice(Y[0][0], j, TL), _fslice(Y[0][1], j, TL)
                y1r, y1i = _fslice(Y[1][0], j, TL), _fslice(Y[1][1], j, TL)
                m1 = scr.tile([128, TL], BF, tag="pp", name="pp", bufs=4)
                m2 = scr.tile([128, TL], BF, tag="pp", name="pp", bufs=4)
                pr = scr.tile([128, TL], BF, tag="pr", name="pr", bufs=2)
                nc.vector.tensor_mul(m1[0:fj, :], y1r, y0r)
                nc.vector.tensor_mul(m2[0:fj, :], y1i, y0i)
                nc.vector.tensor_add(pr[0:fj, :], m1[0:fj, :], m2[0:fj, :])
                m3 = scr.tile([128, TL], BF, tag="pp", name="pp", bufs=4)
                m4 = scr.tile([128, TL], BF, tag="pp", name="pp", bufs=4)
                pi = scr.tile([128, TL], BF, tag="pi", name="pi", bufs=2)
                nc.gpsimd.tensor_mul(m3[0:fj, :], y1i, y0r)
                nc.gpsimd.tensor_mul(m4[0:fj, :], y1r, y0i)
                nc.gpsimd.tensor_sub(pi[0:fj, :], m3[0:fj, :], m4[0:fj, :])
                srcs = [(Wm[0], _fslice(A[0], j, TL), 0),
                        (Wm[1], _fslice(A[0], j, TL), 1),
                        (Wm[0], _fslice(A[1], j, TL), 2),
                        (Wm[1], _fslice(A[1], j, TL), 3),
                        (Wm[0], pr[0:fj, :], 4), (Wm[0], pi[0:fj, :], 5),
                        (Wm[1], pr[0:fj, :], 6), (Wm[1], pi[0:fj, :], 7)]
                for wt, src_ap, q in srcs:
                    prod = scr.tile([128, TL], BF, tag="pd", name="pd", bufs=6)
                    eng = nc.vector if q % 2 == 0 else nc.gpsimd
                    eng.tensor_mul(prod[0:fj, :], _fslice(wt, j, TL), src_ap)
                    nc.vector.tensor_reduce(
                        S[0:fj, q * NJ + j : q * NJ + j + 1], prod[0:fj, :],
                        axis=mybir.AxisListType.X, op=AL.add)

        def allreduce(tile_t, ncols):
            bi = dram.tile([128, ncols], FP, tag="cin", name="cin", bufs=2)
            bo = dram.tile([128, ncols], FP, tag="cout", name="cout", bufs=2)
            nc.sync.dma_start(bi[:], tile_t[:, 0:ncols])
            nc.gpsimd.collective_compute(
                "AllReduce", AL.add,
                replica_groups=cc_pairs,
                ins=[bi.opt()], outs=[bo.opt()])
            nc.sync.dma_start(tile_t[:, 0:ncols], bo[:])

        def smalls():
            """Per-(f) coefficient algebra on [128, NJ] tiles."""
            def t():
                return sm.tile([128, NJ], FP, tag="smt", name="smt")

            def c(name):
                return cf.tile([128, NJ], FP, tag=name, name=name)
            invT = 1.0 / float(T)
            d0, r0 = t(), t()
            alpha = c("alpha")
            nc.vector.tensor_scalar(d0[:], qs(0), invT, EPS, AL.mult, AL.max)
            nc.vector.reciprocal(r0[:], d0[:])
            nc.scalar.activation(alpha[:], r0[:], AF.Sqrt)
            d1, r1 = t(), t()
            nc.vector.tensor_scalar(d1[:], qs(1), EPS, None, AL.max)
            nc.vector.reciprocal(r1[:], d1[:])
            vr = t()
            vi, nvr, nvi = c("vi"), c("nvr"), c("nvi")
            nc.vector.tensor_mul(vr[:], qs(6), r1[:])
            nc.vector.tensor_mul(vi[:], qs(7), r1[:])
            nc.vector.tensor_scalar_mul(nvr[:], vr[:], -1.0)
            nc.vector.tensor_scalar_mul(nvi[:], vi[:], -1.0)
            m2, u = t(), t()
            nc.vector.tensor_mul(m2[:], vr[:], vr[:])
            nc.vector.scalar_tensor_tensor(u[:], vi[:], 1.0, vi[:], AL.mult, AL.mult)
            nc.vector.tensor_add(m2[:], m2[:], u[:])
            # den0' = q2 - 2(vr q4 + vi q5) + m2 q0 ; den1' likewise with q6,q7,q1,q3
            def denp(qa, qb, qden, qs11):
                x1, x2, e = t(), t(), t()
                nc.vector.tensor_mul(x1[:], vr[:], qa)
                nc.vector.scalar_tensor_tensor(x2[:], vi[:], 1.0, qb, AL.mult, AL.mult)
                nc.vector.tensor_add(x1[:], x1[:], x2[:])
                nc.vector.tensor_mul(e[:], m2[:], qden)
                o = t()
                nc.vector.scalar_tensor_tensor(o[:], x1[:], -2.0, qs11, AL.mult, AL.add)
                nc.vector.tensor_add(o[:], o[:], e[:])
                return o
            den0p = denp(qs(4), qs(5), qs(0), qs(2))
            den1p = denp(qs(6), qs(7), qs(1), qs(3))
            dm, rdm = t(), t()
            nc.vector.tensor_scalar(dm[:], den0p[:], EPS, None, AL.max)
            nc.vector.reciprocal(rdm[:], dm[:])
            # v1 = alpha*((q4,-q5) - conj(v) q0) / den0p
            v1r, tA, tB = t(), t(), t()
            v1i, nv1r, nv1i = c("v1i"), c("nv1r"), c("nv1i")
            nc.vector.tensor_mul(tA[:], vr[:], qs(0))
            nc.vector.tensor_sub(tA[:], qs(4), tA[:])
            nc.vector.tensor_mul(tA[:], tA[:], alpha[:])
            nc.vector.tensor_mul(v1r[:], tA[:], rdm[:])
            nc.vector.tensor_mul(tB[:], vi[:], qs(0))
            nc.vector.tensor_sub(tB[:], tB[:], qs(5))
            nc.vector.tensor_mul(tB[:], tB[:], alpha[:])
            nc.vector.tensor_mul(v1i[:], tB[:], rdm[:])
            nc.vector.tensor_scalar_mul(nv1r[:], v1r[:], -1.0)
            nc.vector.tensor_scalar_mul(nv1i[:], v1i[:], -1.0)
            db, rb = t(), t()
            beta = c("beta")
            nc.vector.tensor_scalar(db[:], den1p[:], invT, EPS, AL.mult, AL.max)
            nc.vector.reciprocal(rb[:], db[:])
            nc.scalar.activation(beta[:], rb[:], AF.Sqrt)
            return alpha, beta, vi, nvr, nvi, v1i, nv1r, nv1i

        def apply_phase(alpha, beta, vi, nvr, nvi, v1i, nv1r, nv1i):
            for j in range(NJ):
                fj = FSZ[j]
                y0r, y0i = _fslice(Y[0][0], j, TL), _fslice(Y[0][1], j, TL)
                y1r, y1i = _fslice(Y[1][0], j, TL), _fslice(Y[1][1], j, TL)
                def c_(ct):
                    return ct[0:fj, j : j + 1]
                t1 = scr.tile([128, TL], FP, tag="ap", name="ap", bufs=4)
                y1pr = scr.tile([128, TL], FP, tag="y1p", name="y1p")
                nc.vector.scalar_tensor_tensor(t1[0:fj, :], y0r, c_(nvr), y1r,
                                               AL.mult, AL.add)
                nc.vector.scalar_tensor_tensor(y1pr[0:fj, :], y0i, c_(vi), t1[0:fj, :],
                                               AL.mult, AL.add)
                t2 = scr.tile([128, TL], FP, tag="ap", name="ap", bufs=4)
                y1pi = scr.tile([128, TL], FP, tag="y1p", name="y1p")
                nc.vector.scalar_tensor_tensor(t2[0:fj, :], y0i, c_(nvr), y1i,
                                               AL.mult, AL.add)
                nc.vector.scalar_tensor_tensor(y1pi[0:fj, :], y0r, c_(nvi), t2[0:fj, :],
                                               AL.mult, AL.add)
                s1 = scr.tile([128, TL], FP, tag="ap", name="ap", bufs=4)
                s2 = scr.tile([128, TL], FP, tag="ap", name="ap", bufs=4)
                nc.scalar.mul(s1[0:fj, :], y0r, c_(alpha))
                nc.scalar.mul(s2[0:fj, :], y0i, c_(alpha))
                t3 = scr.tile([128, TL], FP, tag="ap", name="ap", bufs=4)
                nc.vector.scalar_tensor_tensor(t3[0:fj, :], y1pr[0:fj, :], c_(nv1r),
                                               s1[0:fj, :], AL.mult, AL.add)
                nc.vector.scalar_tensor_tensor(y0r, y1pi[0:fj, :], c_(v1i),
                                               t3[0:fj, :], AL.mult, AL.add)
                t4 = scr.tile([128, TL], FP, tag="ap", name="ap", bufs=4)
                nc.vector.scalar_tensor_tensor(t4[0:fj, :], y1pi[0:fj, :], c_(nv1r),
                                               s2[0:fj, :], AL.mult, AL.add)
                nc.vector.scalar_tensor_tensor(y0i, y1pr[0:fj, :], c_(nv1i),
                                               t4[0:fj, :], AL.mult, AL.add)
                nc.scalar.mul(y1r, y1pr[0:fj, :], c_(beta))
                nc.scalar.mul(y1i, y1pi[0:fj, :], c_(beta))

        def mt():
            return sm.tile([128, NJ], FP, tag="mup", name="mup")

        def m_update(alpha, beta, vi, nvr, nvi, v1i, nv1r, nv1i):
            """Accumulate the iteration's 2x2 steering matrix into M.

            Same algebra as apply_phase, applied to each column k of M
            (elementwise on [128, NJ] tiles: one coef per (f, j))."""
            for k in range(2):
                m0r, m0i = Mv(k, 0, 0), Mv(k, 0, 1)
                m1r, m1i = Mv(k, 1, 0), Mv(k, 1, 1)
                t1, t2 = mt(), mt()
                y1pr, y1pi = mt(), mt()
                nc.vector.tensor_mul(t1[:], nvr[:], m0r)
                nc.vector.tensor_add(t1[:], t1[:], m1r)
                nc.vector.tensor_mul(t2[:], vi[:], m0i)
                nc.vector.tensor_add(y1pr[:], t1[:], t2[:])
                t3, t4 = mt(), mt()
                nc.vector.tensor_mul(t3[:], nvr[:], m0i)
                nc.vector.tensor_add(t3[:], t3[:], m1i)
                nc.vector.tensor_mul(t4[:], nvi[:], m0r)
                nc.vector.tensor_add(y1pi[:], t3[:], t4[:])
                s1, u1, u2 = mt(), mt(), mt()
                nc.vector.tensor_mul(s1[:], alpha[:], m0r)
                nc.vector.tensor_mul(u1[:], nv1r[:], y1pr[:])
                nc.vector.tensor_add(s1[:], s1[:], u1[:])
                nc.vector.tensor_mul(u2[:], v1i[:], y1pi[:])
                s2, u3, u4 = mt(), mt(), mt()
                nc.vector.tensor_mul(s2[:], alpha[:], m0i)
                nc.vector.tensor_mul(u3[:], nv1r[:], y1pi[:])
                nc.vector.tensor_add(s2[:], s2[:], u3[:])
                nc.vector.tensor_mul(u4[:], nv1i[:], y1pr[:])
                nc.vector.tensor_add(m0r, s1[:], u2[:])
                nc.vector.tensor_add(m0i, s2[:], u4[:])
                nc.vector.tensor_mul(m1r, beta[:], y1pr[:])
                nc.vector.tensor_mul(m1i, beta[:], y1pi[:])

        # ---- main loop ---------------------------------------------------
        n_it = int(os.environ.get("KITERS", str(N_ITER)))
        do_cc = os.environ.get("KCC", "1") == "1"
        do_pb = os.environ.get("KPB", "1") == "1"
        do_mask = os.environ.get("KMASK", "1") == "1"
        do_stats = os.environ.get("KSTATS", "1") == "1"
        do_apply = os.environ.get("KAPPLY", "1") == "1"
        for _ in range(n_it):
            if do_mask:
                mask_phase()
            if do_stats:
                stats_phase()
            if do_cc:
                allreduce(S, 8 * NJ)
            if do_apply:
                coefs = smalls()
                apply_phase(*coefs)
                m_update(*coefs)

        # ---- projection back: stats over final Y, fold scale into M -----
        for j in ([] if not do_pb else range(NJ)):
            fj = FSZ[j]
            for c in range(C):
                pairs = [(Y[c][0], X0[0]), (Y[c][1], X0[1]),
                         (Y[c][0], X0[1]), (Y[c][1], X0[0]),
                         (Y[c][0], Y[c][0]), (Y[c][1], Y[c][1])]
                for qi, (ta, tb) in enumerate(pairs):
                    q = c * 6 + qi
                    prod = scr.tile([128, TL], FP, tag="pd2", name="pd2", bufs=4)
                    if qi >= 4:
                        nc.scalar.activation(prod[0:fj, :], _fslice(ta, j, TL),
                                             AF.Square)
                    else:
                        eng = nc.vector if qi % 2 == 0 else nc.gpsimd
                        eng.tensor_mul(prod[0:fj, :], _fslice(ta, j, TL),
                                       _fslice(tb, j, TL))
                    nc.vector.tensor_reduce(
                        PB[0:fj, q * NJ + j : q * NJ + j + 1], prod[0:fj, :],
                        axis=mybir.AxisListType.X, op=AL.add)
        if do_pb:
            allreduce(PB, 12 * NJ)

        def pbq(q):
            return PB[:, q * NJ : (q + 1) * NJ]

        for c in ([] if not do_pb else range(C)):
            gq = [pbq(c * 6 + i) for i in range(6)]
            numr = sm.tile([128, NJ], FP, tag="pbs", name="pbs")
            numi = sm.tile([128, NJ], FP, tag="pbs", name="pbs")
            den = sm.tile([128, NJ], FP, tag="pbs", name="pbs")
            rc = sm.tile([128, NJ], FP, tag="pbs", name="pbs")
            cr = sm.tile([128, NJ], FP, tag=f"cr{c}", name=f"cr{c}")
            ci = sm.tile([128, NJ], FP, tag=f"ci{c}", name=f"ci{c}")
            nc.vector.tensor_add(numr[:], gq[0], gq[1])
            nc.vector.tensor_sub(numi[:], gq[2], gq[3])
            nc.vector.tensor_add(den[:], gq[4], gq[5])
            nc.vector.tensor_scalar(den[:], den[:], EPS, None, AL.max)
            nc.vector.reciprocal(rc[:], den[:])
            nc.vector.tensor_mul(cr[:], numr[:], rc[:])
            nc.vector.tensor_mul(ci[:], numi[:], rc[:])
            # M[c, :] *= (cr + i ci)  for both columns k
            for k in range(2):
                mr, mi = Mv(k, c, 0), Mv(k, c, 1)
                u = mt()
                w = mt()
                minew = mt()
                nc.vector.tensor_mul(u[:], cr[:], mi)
                nc.vector.tensor_mul(w[:], ci[:], mr)
                nc.vector.tensor_add(minew[:], u[:], w[:])
                u2, w2 = mt(), mt()
                nc.vector.tensor_mul(u2[:], cr[:], mr)
                nc.vector.tensor_mul(w2[:], ci[:], mi)
                nc.vector.tensor_sub(mr, u2[:], w2[:])
                nc.vector.tensor_copy(mi, minew[:])

        # ---- ship M: gather all cores' M blocks, DMA to output ----------
        mo_i = dram.tile([1, 128 * 8 * NJ], FP, tag="moi", name="moi")
        mo_o = dram.tile([1, OLEN_M], FP, tag="moo", name="moo")
        nc.sync.dma_start(
            mo_i[:].squeeze(0).rearrange("(p k) -> p k", k=8 * NJ), Mt[:])
        nc.gpsimd.collective_compute(
            "AllGather", AL.bypass,
            replica_groups=cc_all,
            ins=[mo_i.opt()], outs=[mo_o.opt()])
        nc.sync.dma_start(po_d.rearrange("(o k) -> o k", o=1), mo_o[:])


class _Exec:
    """Cached jitted shard_map executor: one put to core 0, zero-shard
    dummies for cores 1-7, tiny single-shard M fetch."""

    def __init__(self, nc, devices=None, g=N_CORES):
        import jax
        from jax.sharding import Mesh, PartitionSpec, NamedSharding
        from jax.experimental.shard_map import shard_map
        from concourse.bass2jax import (
            _bass_exec_p, install_neuronx_cc_hook, partition_id_tensor,
        )
        import jax.numpy as jnp

        self.jax = jax
        install_neuronx_cc_hook()
        partition_name = (nc.partition_id_tensor.name
                          if nc.partition_id_tensor else None)
        in_names, out_names, out_avals = [], [], []
        in_len = None
        for alloc in nc.m.functions[0].allocations:
            if not isinstance(alloc, mybir.MemoryLocationSet):
                continue
            name = alloc.memorylocations[0].name
            if alloc.kind == "ExternalInput":
                if name != partition_name:
                    in_names.append(name)
                    in_len = int(alloc.tensor_shape[0])
            elif alloc.kind == "ExternalOutput":
                out_names.append(name)
                out_avals.append(jax.core.ShapedArray(
                    tuple(alloc.tensor_shape), mybir.dt.np(alloc.dtype)))
        assert in_names == ["pk"] and out_names == ["po"], (in_names, out_names)
        assert in_len == PK_H, in_len
        n_params = len(in_names)
        n_outs = len(out_avals)
        all_in = list(in_names) + list(out_names)
        if partition_name is not None:
            all_in.append(partition_name)

        def _bdy(*args):
            operands = list(args)
            if partition_name is not None:
                operands.append(partition_id_tensor())
            return tuple(_bass_exec_p.bind(
                *operands,
                out_avals=tuple(out_avals),
                in_names=tuple(all_in),
                out_names=tuple(out_names),
                lowering_input_output_aliases=(),
                sim_require_finite=True,
                sim_require_nnan=True,
                nc=nc,
            ))

        if devices is None:
            devices = jax.devices()[:g]
        assert len(devices) == g
        self.devices = devices
        self.g = g
        mesh = Mesh(np.asarray(devices), ("core",))
        self.shard = NamedSharding(mesh, PartitionSpec("core"))
        in_specs = (PartitionSpec("core"),) * (n_params + n_outs)
        out_specs = (PartitionSpec("core"),) * n_outs
        jitfn = jax.jit(
            shard_map(_bdy, mesh=mesh, in_specs=in_specs, out_specs=out_specs,
                      check_rep=False),
            donate_argnums=tuple(range(n_params, n_params + n_outs)),
            keep_unused=True,
        )
        aval = out_avals[0]
        gshape = (g * aval.shape[0],)
        gdtype = aval.dtype
        try:
            from concourse.bass2jax import fast_dispatch_compile
            pk_spec = jax.ShapeDtypeStruct((g * PK_H,), np.float16,
                                           sharding=self.shard)
            do_spec = jax.ShapeDtypeStruct(gshape, gdtype, sharding=self.shard)
            self.sharded = fast_dispatch_compile(
                lambda: jitfn.lower(pk_spec, do_spec).compile())
        except Exception:
            self.sharded = jitfn
        self.mkzeros = jax.jit(lambda: jnp.zeros(gshape, gdtype),
                               out_shardings=self.shard)
        # persistent zero input shards for cores 1..7 (never transferred)
        zin = jax.jit(lambda: jnp.zeros((g * PK_H,), jnp.float16),
                      out_shardings=self.shard)()
        zin.block_until_ready()
        self.zsh = [s.data for s in zin.addressable_shards]
        self._zin = zin
        self.prev_out = None
        self.pack_buf = np.empty(PK_B, np.int8)

    def run(self, inputs):
        """Pack + quantize, single device_put to core 0, dispatch, fetch M."""
        jax = self.jax
        donate = self.prev_out if self.prev_out is not None else self.mkzeros()
        dr = np.asarray(inputs["data_real"])
        di = np.asarray(inputs["data_imag"])
        buf = self.pack_buf
        wblob = np.zeros(N_CORES * WSHB, np.uint8)
        wblob[WB_W1 : WB_W2] = np.ascontiguousarray(
            np.asarray(inputs["W1"], np.float32).astype(np.float16)).view(np.uint8).ravel()
        wblob[WB_W2 : WB_B1] = np.ascontiguousarray(
            np.asarray(inputs["W2"], np.float32).astype(np.float16)).view(np.uint8).ravel()
        wblob[WB_B1 : WB_B2] = np.ascontiguousarray(
            np.asarray(inputs["b1"], np.float32)).view(np.uint8).ravel()
        wblob[WB_B2 : WBLOB] = np.ascontiguousarray(
            np.asarray(inputs["b2"], np.float32)).view(np.uint8).ravel()
        wv = wblob.view(np.int8)

        lib = _CACHED.get("clib")
        if lib is not None and dr.flags.c_contiguous and di.flags.c_contiguous \
                and dr.dtype == np.float32 and di.dtype == np.float32:
            import ctypes
            fptr = ctypes.POINTER(ctypes.c_float)
            i8ptr = ctypes.POINTER(ctypes.c_char)
            sco = self.__dict__.setdefault(
                "sctmp", np.empty((B, C, F), np.float32))
            lib.quant_pack(
                dr.ctypes.data_as(fptr), di.ctypes.data_as(fptr),
                buf.ctypes.data_as(i8ptr), sco.ctypes.data_as(fptr),
                ctypes.c_float(QLVL), B, T, C, F,
                ctypes.c_long(SEG_B), ctypes.c_long(NXB),
                ctypes.c_long(OFF_SC))
            for k in range(N_CORES):
                buf[k * SEG_B + OFF_WS : (k + 1) * SEG_B] = \
                    wv[k * WSHB : (k + 1) * WSHB]
        else:
            tmp = self.__dict__.setdefault(
                "qtmp", np.empty((TL, C, F), np.float32))
            q8 = self.__dict__.setdefault(
                "q8tmp", np.empty((TL, C, F), np.int8))
            ab = self.__dict__.setdefault(
                "abtmp", np.empty((T, C, F), np.float32))
            for b in range(B):
                np.abs(dr[b], out=ab)
                a = ab.max(axis=0)
                np.abs(di[b], out=ab)
                np.maximum(a, ab.max(axis=0), out=a)
                np.maximum(a, 1e-20, out=a)
                inv = np.float32(QLVL) / a                  # (C,F)
                sc = (a * np.float32(1.0 / QLVL)).astype(np.float32)
                scv = sc.ravel().view(np.int8)
                for h in range(TSPLIT):
                    k = TSPLIT * b + h
                    seg = buf[k * SEG_B : (k + 1) * SEG_B]
                    sl = slice(h * TL, (h + 1) * TL)
                    for p, src in ((0, dr), (1, di)):
                        np.multiply(src[b, sl], inv, out=tmp)
                        np.rint(tmp, out=tmp)
                        np.copyto(q8, tmp, casting="unsafe")
                        seg[p * NXB : (p + 1) * NXB] = q8.ravel()
                    seg[OFF_SC : OFF_WS] = scv
                    seg[OFF_WS : SEG_B] = wv[k * WSHB : (k + 1) * WSHB]

        s0 = jax.device_put(buf.view(np.float16), self.devices[0])
        arr = jax.make_array_from_single_device_arrays(
            (self.g * PK_H,), self.shard, [s0] + self.zsh[1:])
        (out,) = self.sharded(arr, donate)
        sh0 = out.addressable_shards[0].data
        try:
            sh0.copy_to_host_async()
        except Exception:
            pass
        res = np.asarray(sh0)
        self.prev_out = out
        return res, dr, di


def _unpack_apply(res, dr, di):
    """Decode per-b 2x2 demixing matrices and apply them to the full-
    precision input on the host: out[s,b,t,f] = sum_k M[b,s,k,f] X_k."""
    blocks = res.reshape(N_CORES, 128, 8 * NJ)
    Mk = np.empty((B, 2, 2, 2, F), np.float32)      # [b, k, c, p, f]
    for b in range(B):
        blk = blocks[2 * b]
        mb = Mk[b].reshape(8, F)
        for j in range(NJ):
            fj = FSZ[j]
            mb[:, 128 * j : 128 * j + fj] = blk[0:fj, j::NJ].T
    out = np.empty((2, B, T, F), np.complex64)
    lib = _CACHED.get("clib")
    if lib is not None and dr.flags.c_contiguous and di.flags.c_contiguous \
            and dr.dtype == np.float32 and di.dtype == np.float32:
        import ctypes
        fptr = ctypes.POINTER(ctypes.c_float)
        lib.apply_m(dr.ctypes.data_as(fptr), di.ctypes.data_as(fptr),
                    Mk.ctypes.data_as(fptr),
                    out.view(np.float32).ctypes.data_as(fptr), B, T, C, F)
        return out
    our, oui = out.real, out.imag
    tmp = np.empty((T, F), np.float32)
    for b in range(B):
        xr0, xi0 = dr[b, :, 0, :], di[b, :, 0, :]
        xr1, xi1 = dr[b, :, 1, :], di[b, :, 1, :]
        for s in range(2):
            ar, ai = Mk[b, 0, s, 0][None, :], Mk[b, 0, s, 1][None, :]
            br, bi = Mk[b, 1, s, 0][None, :], Mk[b, 1, s, 1][None, :]
            acc = our[s, b]
            np.multiply(xr0, ar, out=acc)
            np.multiply(xi0, ai, out=tmp)
            np.subtract(acc, tmp, out=acc)
            np.multiply(xr1, br, out=tmp)
            np.add(acc, tmp, out=acc)
            np.multiply(xi1, bi, out=tmp)
            np.subtract(acc, tmp, out=acc)
            acc = oui[s, b]
            np.multiply(xi0, ar, out=acc)
            np.multiply(xr0, ai, out=tmp)
            np.add(acc, tmp, out=acc)
            np.multiply(xi1, br, out=tmp)
            np.add(acc, tmp, out=acc)
            np.multiply(xr1, bi, out=tmp)
            np.add(acc, tmp, out=acc)
    return out


def kernel(**inputs):
    if "ex" not in _CACHED:
        _CACHED["clib"] = _build_chelper()
        _CACHED["nc"] = _build()
        _CACHED["ex"] = _Exec(_CACHED["nc"])
    ex = _CACHED["ex"]
    res, dr, di = ex.run(inputs)
    return _unpack_apply(res, dr, di)


if __name__ == "__main__":
    rng = np.random.default_rng(0)
    ins = {
        "data_real": rng.standard_normal((B, T, C, F), dtype=np.float32),
        "data_imag": rng.standard_normal((B, T, C, F), dtype=np.float32),
        "ilens": np.full((B,), T, dtype=np.int32),
        "W1": rng.standard_normal((F, U), dtype=np.float32) / np.sqrt(F),
        "b1": np.zeros((U,), dtype=np.float32),
        "W2": rng.standard_normal((U, F), dtype=np.float32) / np.sqrt(U),
        "b2": np.zeros((F,), dtype=np.float32),
    }
    out = kernel(**ins)
    print("kernel ran", out.shape, out.dtype, np.abs(out).mean())
